# revision 9
# baseline (speedup 1.0000x reference)
"""Trainium2 Bass kernel for masked general attention (ragged sequences).

reference computation per batch b:
    q       = query[b] @ W_in.T                      [Lq, D]
    S       = q @ context[b].T                       [Lq, Lk]
    S_m     = where(qmask & kmask, S, -1e9)
    W       = softmax(S_m, axis=-1)
    mix     = W @ context[b]                         [Lq, D]
    out     = tanh(concat([mix, q]) @ W_out.T)       [Lq, D]
    returns (out, S_m)

Sharding: data-parallel over batch. 32 batches / 8 cores = 4 per core,
same program on every core (SPMD), weights replicated.

Per-core schedule: 16 q-blocks (4 batches x 4 blocks of 256 queries):
  scores (fp32r) -> min-mask -> softmax -> PE transpose of the weights
  -> mix (fp16) -> output matmul (fp16 mix part + f32r query part) -> tanh.
The q-projection (fp32r) for block g+1 is emitted between scores(g) and
the transposes(g) so the PE stays busy while the softmax chain runs on
the vector/scalar engines.

Masking uses elementwise min against +BIG/-1e9 vectors, which reproduces
the reference's exact -1e9 fill (|scores| < ~1e4 << 1e9) including the
uniform-softmax rows for fully-masked queries.
"""

import sys

sys.path.insert(0, "/opt/trn_rl_repo")

import numpy as np

import concourse.bass as bass
import concourse.tile as tile
from concourse import bacc, mybir
from concourse import bass_utils
from concourse.masks import make_identity

F32 = mybir.dt.float32
F32R = mybir.dt.float32r
FP16 = mybir.dt.float16

B, Lq, Lk, D = 32, 1024, 1024, 1024
N_CORES = 8
BPC = B // N_CORES          # batches per core
MQ = 256                    # queries per block
NBLK = Lq // MQ             # q-blocks per batch
NG = BPC * NBLK             # q-blocks per core
NEG = -1e9
BIG = 3.0e38

_program_cache = {}


def _build_program():
    nc = bacc.Bacc("TRN2", target_bir_lowering=False, debug=False,
                   num_devices=N_CORES)

    # DRAM I/O (per core shard). float32r tensors take np.float32 data.
    qT_d = nc.dram_tensor("qT", [BPC, D, Lq], F32R, kind="ExternalInput").ap()
    cT_d = nc.dram_tensor("cT", [BPC, D, Lk], F32R, kind="ExternalInput").ap()
    cn_d = nc.dram_tensor("cn", [BPC, Lk, D], FP16, kind="ExternalInput").ap()
    winT_d = nc.dram_tensor("winT", [D, D], F32R, kind="ExternalInput").ap()
    wo1_d = nc.dram_tensor("wo1", [D, D], FP16, kind="ExternalInput").ap()
    wo2_d = nc.dram_tensor("wo2", [D, D], F32R, kind="ExternalInput").ap()
    kmin_d = nc.dram_tensor("kmin", [BPC, 128, Lk], F32, kind="ExternalInput").ap()
    qmin_d = nc.dram_tensor("qmin", [BPC, 128, Lq // 128], F32, kind="ExternalInput").ap()

    out_d = nc.dram_tensor("out", [BPC, Lq, D], F32, kind="ExternalOutput").ap()
    sc_d = nc.dram_tensor("sc", [BPC, Lq, Lk], F32, kind="ExternalOutput").ap()

    with tile.TileContext(nc) as tc:
        with (
            tc.tile_pool(name="static", bufs=1) as st,
            tc.tile_pool(name="ctx", bufs=1) as ctx_pool,
            tc.tile_pool(name="qry", bufs=2) as qry_pool,
            tc.tile_pool(name="qTr", bufs=2) as qTr_pool,
            tc.tile_pool(name="mixT", bufs=2) as mixT_pool,
            tc.tile_pool(name="ew", bufs=2) as ew_pool,
            tc.tile_pool(name="wt", bufs=2) as wt_pool,
            tc.tile_pool(name="sm", bufs=4) as sm_pool,
            tc.tile_pool(name="ot", bufs=4) as ot_pool,
            tc.tile_pool(name="stats", bufs=4) as stats_pool,
            tc.tile_pool(name="psA", bufs=2, space="PSUM") as psA,
            tc.tile_pool(name="psS", bufs=2, space="PSUM") as psS,
            tc.tile_pool(name="psO", bufs=2, space="PSUM") as psO,
            tc.tile_pool(name="psT", bufs=2, space="PSUM") as psT,
        ):
            # W_in first: the prologue projection only needs winT + qry(0).
            winT_sb = st.tile([128, 8 * D], F32R, tag="winT")
            for dt in range(8):
                nc.sync.dma_start(winT_sb[:, dt * D:(dt + 1) * D],
                                  winT_d[dt * 128:(dt + 1) * 128, :])

            def stage1_proj(g):
                """qT(g) = W_in @ query-block g (all operands transposed)."""
                b, blk_i = divmod(g, NBLK)
                q0 = blk_i * MQ
                qry_sb = qry_pool.tile([128, 8 * MQ], F32R, tag="qry")
                for dt in range(8):
                    nc.sync.dma_start(
                        qry_sb[:, dt * MQ:(dt + 1) * MQ],
                        qT_d[b, dt * 128:(dt + 1) * 128, q0:q0 + MQ])
                qTr_sb = qTr_pool.tile([128, 8 * MQ], F32R, tag="qTr")
                for et in range(8):
                    pq = psA.tile([128, MQ], F32, tag="psA")
                    for dt in range(8):
                        nc.tensor.matmul(
                            pq[:],
                            winT_sb[:, dt * D + et * 128:dt * D + (et + 1) * 128],
                            qry_sb[:, dt * MQ:(dt + 1) * MQ],
                            start=(dt == 0), stop=(dt == 7))
                    nc.vector.tensor_copy(qTr_sb[:, et * MQ:(et + 1) * MQ], pq[:])
                return qTr_sb

            def load_ctx(b):
                """Per-batch context (both layouts) + masks."""
                cT_sb = ctx_pool.tile([128, 8 * Lk], F32R, tag="cT")
                for et in range(8):
                    nc.sync.dma_start(cT_sb[:, et * Lk:(et + 1) * Lk],
                                      cT_d[b, et * 128:(et + 1) * 128, :])
                cn_sb = ctx_pool.tile([128, 8 * D], FP16, tag="cn")
                for kt in range(8):
                    nc.sync.dma_start(cn_sb[:, kt * D:(kt + 1) * D],
                                      cn_d[b, kt * 128:(kt + 1) * 128, :])
                kmin_sb = ctx_pool.tile([128, Lk], F32, tag="kmin")
                nc.sync.dma_start(kmin_sb[:], kmin_d[b])
                qmin_sb = ctx_pool.tile([128, Lq // 128], F32, tag="qmin")
                nc.sync.dma_start(qmin_sb[:], qmin_d[b])
                return cT_sb, cn_sb, kmin_sb, qmin_sb

            # ---- prologue: project block 0, then the deferred statics -----
            qTr_cur = stage1_proj(0)
            ctx_cur = load_ctx(0)

            wo1_sb = st.tile([128, 8 * D], FP16, tag="wo1")
            for ct in range(8):
                nc.sync.dma_start(wo1_sb[:, ct * D:(ct + 1) * D],
                                  wo1_d[ct * 128:(ct + 1) * 128, :])
            wo2_sb = st.tile([128, 8 * D], F32R, tag="wo2")
            for ct in range(8):
                nc.sync.dma_start(wo2_sb[:, ct * D:(ct + 1) * D],
                                  wo2_d[ct * 128:(ct + 1) * 128, :])
            ident = st.tile([128, 128], FP16, tag="ident")
            make_identity(nc, ident[:])

            for g in range(NG):
                b, blk_i = divmod(g, NBLK)
                q0 = blk_i * MQ
                cT_sb, cn_sb, kmin_sb, qmin_sb = ctx_cur
                qTr_sb = qTr_cur

                # ---- stage 2: scores + mask + softmax (no transposes) -----
                ew_sb = ew_pool.tile([128, 2 * Lk], FP16, tag="ew")
                for h in range(2):
                    jt = blk_i * 2 + h          # q-tile index within batch
                    rows = slice(q0 + h * 128, q0 + (h + 1) * 128)
                    stt = stats_pool.tile([128, 8], F32, tag="stats")
                    sm_n = []
                    for n in range(2):
                        ps = psS.tile([128, 512], F32, tag="psS")
                        for et in range(8):
                            nc.tensor.matmul(
                                ps[:],
                                qTr_sb[:, et * MQ + h * 128:et * MQ + (h + 1) * 128],
                                cT_sb[:, et * Lk + n * 512:et * Lk + (n + 1) * 512],
                                start=(et == 0), stop=(et == 7))
                        # k-mask then q-mask (exact -1e9 fill via min)
                        sm = sm_pool.tile([128, 512], F32, tag="sm")
                        sm_n.append(sm)
                        nc.vector.tensor_tensor(
                            sm[:], ps[:], kmin_sb[:, n * 512:(n + 1) * 512],
                            op=mybir.AluOpType.min)
                        nc.vector.tensor_scalar_min(
                            sm[:], sm[:], qmin_sb[:, jt:jt + 1])
                        nc.sync.dma_start(sc_d[b, rows, n * 512:(n + 1) * 512],
                                          sm[:])
                        nc.vector.reduce_max(
                            stt[:, n:n + 1], sm[:],
                            axis=mybir.AxisListType.X, negate=True)
                    # -max over the full row; exp both halves with row-sums
                    nc.vector.tensor_tensor(
                        stt[:, 2:3], stt[:, 0:1], stt[:, 1:2],
                        op=mybir.AluOpType.min)
                    for n in range(2):
                        nc.scalar.activation(
                            ew_sb[:, h * Lk + n * 512:h * Lk + (n + 1) * 512],
                            sm_n[n][:],
                            mybir.ActivationFunctionType.Exp,
                            bias=stt[:, 2:3], scale=1.0,
                            accum_out=stt[:, 3 + n:4 + n])
                    nc.vector.tensor_tensor(
                        stt[:, 5:6], stt[:, 3:4], stt[:, 4:5],
                        op=mybir.AluOpType.add)
                    nc.vector.reciprocal(stt[:, 6:7], stt[:, 5:6])
                    nc.vector.tensor_scalar_mul(
                        ew_sb[:, h * Lk:(h + 1) * Lk],
                        ew_sb[:, h * Lk:(h + 1) * Lk],
                        stt[:, 6:7])

                # ---- stage 1 of block g+1 fills the PE while softmax runs -
                if g + 1 < NG:
                    qTr_next = stage1_proj(g + 1)
                    nb = (g + 1) // NBLK
                    ctx_next = load_ctx(nb) if nb != b else ctx_cur
                else:
                    qTr_next, ctx_next = None, None

                # ---- transposes: W [128q,128k] tiles -> Wt [128k,256q] ----
                wt_sb = wt_pool.tile([128, 8 * MQ], FP16, tag="wt")
                for kt in range(8):
                    pt = psT.tile([128, MQ], FP16, tag="psT")
                    for h in range(2):
                        nc.tensor.transpose(
                            pt[:, h * 128:(h + 1) * 128],
                            ew_sb[:, h * Lk + kt * 128:h * Lk + (kt + 1) * 128],
                            ident[:])
                    nc.vector.tensor_copy(wt_sb[:, kt * MQ:(kt + 1) * MQ], pt[:])

                # ---- stage 3: mixT = context.T @ W.T ----------------------
                mixT_sb = mixT_pool.tile([128, 8 * MQ], FP16, tag="mixT")
                for dt in range(8):
                    pm = psA.tile([128, MQ], F32, tag="psA")
                    for kt in range(8):
                        nc.tensor.matmul(
                            pm[:],
                            cn_sb[:, kt * D + dt * 128:kt * D + (dt + 1) * 128],
                            wt_sb[:, kt * MQ:(kt + 1) * MQ],
                            start=(kt == 0), stop=(kt == 7))
                    nc.vector.tensor_copy(mixT_sb[:, dt * MQ:(dt + 1) * MQ], pm[:])

                # ---- stage 4: out = tanh([mix, q] @ W_out.T) --------------
                for h in range(2):
                    rows = slice(q0 + h * 128, q0 + (h + 1) * 128)
                    for n in range(2):
                        po = psO.tile([128, 512], F32, tag="psO")
                        for dt in range(8):
                            nc.tensor.matmul(
                                po[:],
                                mixT_sb[:, dt * MQ + h * 128:dt * MQ + (h + 1) * 128],
                                wo1_sb[:, dt * D + n * 512:dt * D + (n + 1) * 512],
                                start=(dt == 0), stop=False)
                        for et in range(8):
                            nc.tensor.matmul(
                                po[:],
                                qTr_sb[:, et * MQ + h * 128:et * MQ + (h + 1) * 128],
                                wo2_sb[:, et * D + n * 512:et * D + (n + 1) * 512],
                                start=False, stop=(et == 7))
                        ot = ot_pool.tile([128, 512], F32, tag="ot")
                        nc.scalar.activation(
                            ot[:], po[:], mybir.ActivationFunctionType.Tanh)
                        nc.sync.dma_start(out_d[b, rows, n * 512:(n + 1) * 512],
                                          ot[:])

                qTr_cur, ctx_cur = qTr_next, ctx_next

    nc.compile()
    return nc


def _get_program():
    if "nc" not in _program_cache:
        _program_cache["nc"] = _build_program()
    return _program_cache["nc"]


def kernel(query, context, query_lengths, context_lengths, W_in, W_out):
    nc = _get_program()

    qT = np.ascontiguousarray(query.transpose(0, 2, 1), dtype=np.float32)
    cT = np.ascontiguousarray(context.transpose(0, 2, 1), dtype=np.float32)
    cn = np.ascontiguousarray(context, dtype=np.float32).astype(np.float16)
    winT = np.ascontiguousarray(W_in.T, dtype=np.float32)
    woT = np.ascontiguousarray(W_out.T, dtype=np.float32)
    wo1 = woT[:D].astype(np.float16)
    wo2 = np.ascontiguousarray(woT[D:])

    k_idx = np.arange(Lk, dtype=np.int64)
    q_idx = np.arange(Lq, dtype=np.int64)
    kmin = np.where(k_idx[None, :] < context_lengths[:, None].astype(np.int64),
                    np.float32(BIG), np.float32(NEG)).astype(np.float32)  # [B, Lk]
    qmin = np.where(q_idx[None, :] < query_lengths[:, None].astype(np.int64),
                    np.float32(BIG), np.float32(NEG)).astype(np.float32)  # [B, Lq]
    kmin_rep = np.ascontiguousarray(
        np.broadcast_to(kmin[:, None, :], (B, 128, Lk)), dtype=np.float32)
    qmin_til = np.ascontiguousarray(
        qmin.reshape(B, Lq // 128, 128).transpose(0, 2, 1), dtype=np.float32)

    in_maps = []
    for c in range(N_CORES):
        s = slice(c * BPC, (c + 1) * BPC)
        in_maps.append({
            "qT": qT[s], "cT": cT[s], "cn": cn[s],
            "winT": winT, "wo1": wo1, "wo2": wo2,
            "kmin": kmin_rep[s], "qmin": qmin_til[s],
        })

    res = bass_utils.run_bass_kernel_spmd(nc, in_maps, core_ids=list(range(N_CORES)))
    _program_cache["last_result"] = res

    out = np.concatenate([res.results[c]["out"] for c in range(N_CORES)], axis=0)
    scores = np.concatenate([res.results[c]["sc"] for c in range(N_CORES)], axis=0)
    return out, scores


# revision 17
# speedup vs baseline: 1.4323x; 1.4323x over previous
"""Trainium2 Bass kernel for masked general attention (ragged sequences).

reference computation per batch b:
    q       = query[b] @ W_in.T                      [Lq, D]
    S       = q @ context[b].T                       [Lq, Lk]
    S_m     = where(qmask & kmask, S, -1e9)
    W       = softmax(S_m, axis=-1)
    mix     = W @ context[b]                         [Lq, D]
    out     = tanh(concat([mix, q]) @ W_out.T)       [Lq, D]
    returns (out, S_m)

Sharding / specialization strategy:
- Data-parallel over batch: 32 batches / 8 cores, SPMD (one program).
- W_in is folded away on the host: scores = query @ (context @ W_in)^T
  via an on-device per-batch projection of the context (cw), and the
  query half of the output matmul uses Wfused = W_in.T @ W_out[:,D:].T
  so the per-block q-projection disappears entirely.
- Ragged-length specialization: batches are assigned to 4 "slots" (one
  batch per slot per core) minimizing the baked cost; per slot the
  program only computes attention for q-tiles below the slot's max
  query length and k-tiles below the slot's max context length.
  Skipped score regions are filled with the exact -1e9 constant; rows
  of fully-masked queries get uniform-softmax semantics via a rank-1
  (context-mean x masked-q-indicator) correction added into the mix
  accumulation, matching the reference bit-for-bit in structure.

dtypes: scores chain in float32r (TF32-class), mix/out matmuls in fp16,
softmax stats in fp32. Masking uses elementwise min against +BIG/-1e9
vectors, which reproduces the reference's exact -1e9 fill.
"""

import sys

sys.path.insert(0, "/opt/trn_rl_repo")

import random

import numpy as np
import ml_dtypes

import concourse.bass as bass
import concourse.tile as tile
from concourse import bacc, mybir
from concourse import bass_utils
from concourse.masks import make_identity

F32 = mybir.dt.float32
F32R = mybir.dt.float32r
FP16 = mybir.dt.float16
BF16 = mybir.dt.bfloat16

B, Lq, Lk, D = 32, 1024, 1024, 1024
N_CORES = 8
BPC = B // N_CORES          # batches (slots) per core
MQ = 256                    # queries per block
NBLK = Lq // MQ             # q-blocks per batch
NEG = -1e9
BIG = 3.0e38

_cache = {}


def _slot_cost(nq_max, nk_max):
    nqb = (nq_max + 1) // 2
    nkn = (nk_max + 3) // 4
    cw = nkn * 32768
    comp = nqb * (nkn * 8192 + nk_max * (256 + 2048) + 32768)
    skip = (4 - nqb) * 16384
    return cw + comp + skip


def _assign_slots(query_lengths, context_lengths):
    """Partition the 32 batches into 4 slots x 8 cores minimizing the
    baked per-slot cost. Returns perm[slot][core] -> batch index and the
    per-slot (NQB, NKN, NKT)."""
    nqt = -(-query_lengths.astype(np.int64) // 128)
    nkt = -(-context_lengths.astype(np.int64) // 128)
    order = np.argsort(nqt * nkt)
    slots = [list(order[j * N_CORES:(j + 1) * N_CORES]) for j in range(BPC)]

    def total(ss):
        return sum(_slot_cost(max(nqt[i] for i in s), max(nkt[i] for i in s))
                   for s in ss)

    best = total(slots)
    rng = random.Random(0)
    for _ in range(60000):
        a, b = rng.randrange(BPC), rng.randrange(BPC)
        if a == b:
            continue
        i, j = rng.randrange(N_CORES), rng.randrange(N_CORES)
        slots[a][i], slots[b][j] = slots[b][j], slots[a][i]
        c = total(slots)
        if c <= best:
            best = c
        else:
            slots[a][i], slots[b][j] = slots[b][j], slots[a][i]

    params = []
    for s in slots:
        nq, nk = max(nqt[i] for i in s), max(nkt[i] for i in s)
        params.append((int((nq + 1) // 2), int((nk + 3) // 4), int(nk)))
    return slots, tuple(params)


def _build_program(params):
    """params: tuple of (NQB, NKN, NKT) per slot."""
    nc = bacc.Bacc("TRN2", target_bir_lowering=False, debug=False,
                   num_devices=N_CORES)

    qT_d = nc.dram_tensor("qT", [BPC, D, Lq], F32R, kind="ExternalInput").ap()
    qT16_d = nc.dram_tensor("qT16", [BPC, D, Lq], FP16, kind="ExternalInput").ap()
    cT_d = nc.dram_tensor("cT", [BPC, D, Lk], F32R, kind="ExternalInput").ap()
    cn_d = nc.dram_tensor("cn", [BPC, Lk, D], FP16, kind="ExternalInput").ap()
    win_d = nc.dram_tensor("win", [D, D], F32R, kind="ExternalInput").ap()
    wo1_d = nc.dram_tensor("wo1", [D, D], FP16, kind="ExternalInput").ap()
    wf_d = nc.dram_tensor("wf", [D, D], FP16, kind="ExternalInput").ap()
    kmin_d = nc.dram_tensor("kmin", [BPC, 128, Lk], F32, kind="ExternalInput").ap()
    qmin_d = nc.dram_tensor("qmin", [BPC, 128, Lq // 128], F32, kind="ExternalInput").ap()
    q01_d = nc.dram_tensor("q01", [BPC, 128, Lq // 128], F32, kind="ExternalInput").ap()
    m01_d = nc.dram_tensor("m01", [BPC, Lq], FP16, kind="ExternalInput").ap()
    mean_d = nc.dram_tensor("mean", [BPC, D], FP16, kind="ExternalInput").ap()
    cb_d = nc.dram_tensor("cb", [BPC, D], FP16, kind="ExternalInput").ap()

    out_d = nc.dram_tensor("out", [BPC, Lq, D], F32, kind="ExternalOutput").ap()
    sc_d = nc.dram_tensor("sc", [BPC, Lq, Lk], F32, kind="ExternalOutput").ap()

    with tile.TileContext(nc) as tc:
        with (
            tc.tile_pool(name="static", bufs=1) as st,
            tc.tile_pool(name="ctx", bufs=1) as ctx_pool,
            tc.tile_pool(name="qry", bufs=2) as qry_pool,
            tc.tile_pool(name="q16", bufs=1) as q16_pool,
            tc.tile_pool(name="q16s", bufs=1) as q16s_pool,
            tc.tile_pool(name="ew", bufs=2) as ew_pool,
            tc.tile_pool(name="wm", bufs=2) as wm_pool,
            tc.tile_pool(name="sm", bufs=2) as sm_pool,
            tc.tile_pool(name="ot", bufs=2) as ot_pool,
            tc.tile_pool(name="stats", bufs=4) as stats_pool,
            tc.tile_pool(name="psA", bufs=2, space="PSUM") as psA,
            tc.tile_pool(name="psS", bufs=2, space="PSUM") as psS,
            tc.tile_pool(name="psO", bufs=2, space="PSUM") as psO,
            tc.tile_pool(name="psT", bufs=2, space="PSUM") as psT,
        ):
            win_sb = st.tile([128, 8 * D], F32R, tag="win")
            for et in range(8):
                nc.sync.dma_start(win_sb[:, et * D:(et + 1) * D],
                                  win_d[et * 128:(et + 1) * 128, :])

            def qry_dma(b, blk_i):
                q0 = blk_i * MQ
                t = qry_pool.tile([128, 8 * MQ], F32R, tag="qry")
                for dt in range(8):
                    nc.sync.dma_start(
                        t[:, dt * MQ:(dt + 1) * MQ],
                        qT_d[b, dt * 128:(dt + 1) * 128, q0:q0 + MQ])
                return t

            def qry16_dma(b, blk_i, pool=None, tag="q16"):
                q0 = blk_i * MQ
                pool = pool or q16_pool
                t = pool.tile([128, 8 * MQ], FP16, tag=tag)
                for dt in range(8):
                    nc.sync.dma_start(
                        t[:, dt * MQ:(dt + 1) * MQ],
                        qT16_d[b, dt * 128:(dt + 1) * 128, q0:q0 + MQ])
                return t

            def load_ctx_early(b):
                """Tiles needed by cw/scores: safe to DMA as soon as the
                previous batch's scores are done."""
                NQB, NKN, NKT = params[b]
                cT_sb = ctx_pool.tile([128, 8 * Lk], F32R, tag="cT")
                for et in range(8):
                    nc.sync.dma_start(
                        cT_sb[:, et * Lk:et * Lk + NKN * 512],
                        cT_d[b, et * 128:(et + 1) * 128, :NKN * 512])
                kmin_sb = ctx_pool.tile([128, Lk], F32, tag="kmin")
                nc.sync.dma_start(kmin_sb[:, :NKN * 512], kmin_d[b, :, :NKN * 512])
                qmin_sb = ctx_pool.tile([128, Lq // 128], F32, tag="qmin")
                nc.sync.dma_start(qmin_sb[:], qmin_d[b])
                q01_sb = ctx_pool.tile([128, Lq // 128], F32, tag="q01")
                nc.sync.dma_start(q01_sb[:], q01_d[b])
                return dict(cT=cT_sb, kmin=kmin_sb, qmin=qmin_sb, q01=q01_sb)

            def load_ctx_late(b, ctx):
                """Tiles whose slots are released only by the previous
                batch's last mix: emitted after that mix to avoid blocking
                the in-order DMA queue behind an unsatisfiable wait."""
                NQB, NKN, NKT = params[b]
                cn_sb = ctx_pool.tile([128, 8 * D], FP16, tag="cn")
                for kt in range(NKT):
                    nc.sync.dma_start(cn_sb[:, kt * D:(kt + 1) * D],
                                      cn_d[b, kt * 128:(kt + 1) * 128, :])
                m01_sb = ctx_pool.tile([1, Lq], FP16, tag="m01")
                nc.sync.dma_start(m01_sb[:], m01_d[b:b + 1, :])
                mean_sb = ctx_pool.tile([1, D], FP16, tag="mean")
                nc.sync.dma_start(mean_sb[:], mean_d[b:b + 1, :])
                cb_sb = ctx_pool.tile([1, D], FP16, tag="cb")
                nc.sync.dma_start(cb_sb[:], cb_d[b:b + 1, :])
                ctx.update(cn=cn_sb, m01=m01_sb, mean=mean_sb, cb=cb_sb)

            def cw_build(b, ctx):
                """cw[d, k] = sum_e W_in[e, d] * contextT[e, k] (f32r)."""
                NQB, NKN, NKT = params[b]
                cw_sb = ctx_pool.tile([128, 8 * Lk], F32R, tag="cw")
                for dt in range(8):
                    for n in range(NKN):
                        ps = psS.tile([128, 512], F32, tag="psS")
                        for et in range(8):
                            nc.tensor.matmul(
                                ps[:],
                                win_sb[:, et * D + dt * 128:et * D + (dt + 1) * 128],
                                ctx["cT"][:, et * Lk + n * 512:et * Lk + (n + 1) * 512],
                                start=(et == 0), stop=(et == 7))
                        nc.vector.tensor_copy(
                            cw_sb[:, dt * Lk + n * 512:dt * Lk + (n + 1) * 512],
                            ps[:])
                ctx["cw"] = cw_sb

            # one [128, 1024] tile of exact -1e9 for skipped score regions,
            # plus a [1, 128] ones row for the constant-output rank-1 matmul
            const_sb = st.tile([128, 512], F32, tag="const")
            nc.vector.memset(const_sb[:], NEG)
            ones_sb = st.tile([1, 128], FP16, tag="ones")
            nc.vector.memset(ones_sb[:], 1.0)

            def scores_softmax(b, blk_i, qry_sb, ctx):
                """Masked scores -> DRAM; softmax weights -> ew tile."""
                NQB, NKN, NKT = params[b]
                q0 = blk_i * MQ
                ew_sb = ew_pool.tile([128, 2 * Lk], FP16, tag="ew")
                for h in range(2):
                    jt = blk_i * 2 + h
                    rows = slice(q0 + h * 128, q0 + (h + 1) * 128)
                    stt = stats_pool.tile([128, 8], F32, tag="stats")
                    sm_n = []
                    for n in range(NKN):
                        ps = psS.tile([128, 512], F32, tag="psS")
                        for dt in range(8):
                            nc.tensor.matmul(
                                ps[:],
                                qry_sb[:, dt * MQ + h * 128:dt * MQ + (h + 1) * 128],
                                ctx["cw"][:, dt * Lk + n * 512:dt * Lk + (n + 1) * 512],
                                start=(dt == 0), stop=(dt == 7))
                        sm = sm_pool.tile([128, 512], F32, tag="sm")
                        sm_n.append(sm)
                        nc.vector.tensor_tensor(
                            sm[:], ps[:], ctx["kmin"][:, n * 512:(n + 1) * 512],
                            op=mybir.AluOpType.min)
                        nc.vector.tensor_scalar_min(
                            sm[:], sm[:], ctx["qmin"][:, jt:jt + 1])
                        nc.sync.dma_start(sc_d[b, rows, n * 512:(n + 1) * 512],
                                          sm[:])
                        nc.vector.reduce_max(
                            stt[:, n:n + 1], sm[:],
                            axis=mybir.AxisListType.X, negate=True)
                    if NKN == 1:
                        nc.sync.dma_start(sc_d[b, rows, 512:], const_sb[:])
                        negm = stt[:, 0:1]
                    else:
                        nc.vector.tensor_tensor(
                            stt[:, 2:3], stt[:, 0:1], stt[:, 1:2],
                            op=mybir.AluOpType.min)
                        negm = stt[:, 2:3]
                    for n in range(NKN):
                        nc.scalar.activation(
                            ew_sb[:, h * Lk + n * 512:h * Lk + (n + 1) * 512],
                            sm_n[n][:],
                            mybir.ActivationFunctionType.Exp,
                            bias=negm, scale=1.0,
                            accum_out=stt[:, 3 + n:4 + n])
                    if NKN == 1:
                        ssum = stt[:, 3:4]
                    else:
                        nc.vector.tensor_tensor(
                            stt[:, 5:6], stt[:, 3:4], stt[:, 4:5],
                            op=mybir.AluOpType.add)
                        ssum = stt[:, 5:6]
                    nc.vector.reciprocal(stt[:, 6:7], ssum)
                    if NKT == 8:
                        scale = stt[:, 6:7]
                    else:
                        # zero the weights of fully-masked query rows; their
                        # uniform mix is re-added as a rank-1 term in stage 3
                        nc.vector.tensor_tensor(
                            stt[:, 7:8], stt[:, 6:7], ctx["q01"][:, jt:jt + 1],
                            op=mybir.AluOpType.mult)
                        scale = stt[:, 7:8]
                    nc.vector.tensor_scalar_mul(
                        ew_sb[:, h * Lk:h * Lk + NKN * 512],
                        ew_sb[:, h * Lk:h * Lk + NKN * 512],
                        scale)
                return ew_sb

            def transposes(b, ew_sb, ident):
                NQB, NKN, NKT = params[b]
                wt_sb = wm_pool.tile([128, 8 * MQ], FP16, tag="wm")
                for kt in range(NKT):
                    pt = psT.tile([128, MQ], FP16, tag="psT")
                    for h in range(2):
                        nc.tensor.transpose(
                            pt[:, h * 128:(h + 1) * 128],
                            ew_sb[:, h * Lk + kt * 128:h * Lk + (kt + 1) * 128],
                            ident[:])
                    nc.vector.tensor_copy(wt_sb[:, kt * MQ:(kt + 1) * MQ], pt[:])
                return wt_sb

            def mix_stage(b, blk_i, wt_sb, ctx):
                NQB, NKN, NKT = params[b]
                q0 = blk_i * MQ
                mixT_sb = wm_pool.tile([128, 8 * MQ], FP16, tag="wm")
                for dt in range(8):
                    pm = psA.tile([128, MQ], F32, tag="psA")
                    for kt in range(NKT):
                        nc.tensor.matmul(
                            pm[:],
                            ctx["cn"][:, kt * D + dt * 128:kt * D + (dt + 1) * 128],
                            wt_sb[:, kt * MQ:(kt + 1) * MQ],
                            start=(kt == 0), stop=(kt == NKT - 1 and NKT == 8))
                        # rank-1: uniform context-mean for fully-masked queries
                    if NKT < 8:
                        nc.tensor.matmul(
                            pm[:],
                            ctx["mean"][0:1, dt * 128:(dt + 1) * 128],
                            ctx["m01"][0:1, q0:q0 + MQ],
                            start=False, stop=True)
                    nc.vector.tensor_copy(mixT_sb[:, dt * MQ:(dt + 1) * MQ], pm[:])
                return mixT_sb

            def out_stage(b, blk_i, q16_sb, mixT_sb):
                q0 = blk_i * MQ
                for h in range(2):
                    rows = slice(q0 + h * 128, q0 + (h + 1) * 128)
                    for n in range(2):
                        po = psO.tile([128, 512], F32, tag="psO")
                        for dt in range(8):
                            nc.tensor.matmul(
                                po[:],
                                mixT_sb[:, dt * MQ + h * 128:dt * MQ + (h + 1) * 128],
                                wo1_sb[:, dt * D + n * 512:dt * D + (n + 1) * 512],
                                start=(dt == 0), stop=False)
                        for dt in range(8):
                            nc.tensor.matmul(
                                po[:],
                                q16_sb[:, dt * MQ + h * 128:dt * MQ + (h + 1) * 128],
                                wf_sb[:, dt * D + n * 512:dt * D + (n + 1) * 512],
                                start=False, stop=(dt == 7))
                        ot = ot_pool.tile([128, 512], F32, tag="ot")
                        nc.scalar.activation(
                            ot[:], po[:], mybir.ActivationFunctionType.Tanh)
                        nc.sync.dma_start(out_d[b, rows, n * 512:(n + 1) * 512],
                                          ot[:])

            def skipped_block(b, blk_i, ctx):
                """q-block past every query length in the slot: scores are
                all -1e9; out = tanh(query@Wfused + mean_ctx@Wo1)."""
                q0 = blk_i * MQ
                q16_sb = qry16_dma(b, blk_i, pool=q16s_pool, tag="q16s")
                for h in range(2):
                    rows = slice(q0 + h * 128, q0 + (h + 1) * 128)
                    for n4 in range(2):
                        nc.sync.dma_start(
                            sc_d[b, rows, n4 * 512:(n4 + 1) * 512], const_sb[:])
                    for n in range(2):
                        po = psO.tile([128, 512], F32, tag="psO")
                        for dt in range(8):
                            nc.tensor.matmul(
                                po[:],
                                q16_sb[:, dt * MQ + h * 128:dt * MQ + (h + 1) * 128],
                                wf_sb[:, dt * D + n * 512:dt * D + (n + 1) * 512],
                                start=(dt == 0), stop=False)
                        nc.tensor.matmul(
                            po[:], ones_sb[0:1, :],
                            ctx["cb"][0:1, n * 512:(n + 1) * 512],
                            start=False, stop=True)
                        ot = ot_pool.tile([128, 512], F32, tag="ot")
                        nc.scalar.activation(
                            ot[:], po[:], mybir.ActivationFunctionType.Tanh)
                        nc.sync.dma_start(out_d[b, rows, n * 512:(n + 1) * 512],
                                          ot[:])

            # ---- prologue ---------------------------------------------
            qry0 = qry_dma(0, 0)
            ctx0 = load_ctx_early(0)
            load_ctx_late(0, ctx0)

            wo1_sb = st.tile([128, 8 * D], FP16, tag="wo1")
            for ct in range(8):
                nc.sync.dma_start(wo1_sb[:, ct * D:(ct + 1) * D],
                                  wo1_d[ct * 128:(ct + 1) * 128, :])
            wf_sb = st.tile([128, 8 * D], FP16, tag="wf")
            for ct in range(8):
                nc.sync.dma_start(wf_sb[:, ct * D:(ct + 1) * D],
                                  wf_d[ct * 128:(ct + 1) * 128, :])
            ident = st.tile([128, 128], FP16, tag="ident")
            make_identity(nc, ident[:])

            cw_build(0, ctx0)

            # flattened computed-block sequence with one-block lookahead:
            # scores(next) is emitted before transposes(cur) so the PE has
            # work while the softmax chain runs on DVE/ACT.
            seq = [(b, i) for b in range(BPC) for i in range(params[b][0])]
            cur_ctx = {0: ctx0}
            q16_0 = qry16_dma(0, 0)
            pend = (0, 0, q16_0, scores_softmax(0, 0, qry0, ctx0))
            for idx in range(len(seq)):
                b, i = seq[idx]
                _, _, q16_sb, ew_sb = pend
                ctx = cur_ctx[b]
                nxt = seq[idx + 1] if idx + 1 < len(seq) else None
                if nxt is not None:
                    nb, ni = nxt
                    if nb != b:
                        nctx = load_ctx_early(nb)
                        cw_build(nb, nctx)
                        cur_ctx[nb] = nctx
                    nqry = qry_dma(nb, ni)
                    sew = scores_softmax(nb, ni, nqry, cur_ctx[nb])
                    pend = (nb, ni, qry16_dma(nb, ni), sew)
                wt_sb = transposes(b, ew_sb, ident)
                mixT_sb = mix_stage(b, i, wt_sb, ctx)
                out_stage(b, i, q16_sb, mixT_sb)
                if nxt is None or nxt[0] != b:
                    for si in range(params[b][0], NBLK):
                        skipped_block(b, si, ctx)
                if nxt is not None and nxt[0] != b:
                    load_ctx_late(nxt[0], cur_ctx[nxt[0]])

    nc.compile()
    return nc


def kernel(query, context, query_lengths, context_lengths, W_in, W_out):
    slots, params = _assign_slots(np.asarray(query_lengths),
                                  np.asarray(context_lengths))
    if _cache.get("params") != params:
        _cache["nc"] = _build_program(params)
        _cache["params"] = params
    nc = _cache["nc"]

    # batch order: core c processes batches [slots[0][c], slots[1][c], ...]
    perm = np.array(slots)                       # [BPC, N_CORES]
    flat = perm.T.reshape(-1)                    # core-major batch order

    query = np.asarray(query, dtype=np.float32)
    context = np.asarray(context, dtype=np.float32)
    ql = np.asarray(query_lengths).astype(np.int64)
    cl = np.asarray(context_lengths).astype(np.int64)

    qT = np.ascontiguousarray(query.transpose(0, 2, 1))
    qT16 = qT.astype(np.float16)
    cT = np.ascontiguousarray(context.transpose(0, 2, 1))
    cn = context.astype(np.float16)
    win = np.ascontiguousarray(W_in, dtype=np.float32)
    woT = np.ascontiguousarray(W_out.T, dtype=np.float32)
    wo1 = woT[:D].astype(np.float16)
    wf = (W_in.astype(np.float64).T @ woT[D:].astype(np.float64)).astype(np.float16)
    mean_c = context.astype(np.float64).mean(axis=1)           # [B, D]
    cb = (mean_c @ woT[:D].astype(np.float64)).astype(np.float16)
    mean_c = mean_c.astype(np.float16)

    k_idx = np.arange(Lk)
    q_idx = np.arange(Lq)
    kvalid = k_idx[None, :] < cl[:, None]
    qvalid = q_idx[None, :] < ql[:, None]
    kmin = np.where(kvalid, np.float32(BIG), np.float32(NEG)).astype(np.float32)
    qmin = np.where(qvalid, np.float32(BIG), np.float32(NEG)).astype(np.float32)
    q01 = qvalid.astype(np.float32)
    m01 = (~qvalid).astype(np.float16)
    kmin_rep = np.ascontiguousarray(
        np.broadcast_to(kmin[:, None, :], (B, 128, Lk)))
    qmin_til = np.ascontiguousarray(
        qmin.reshape(B, Lq // 128, 128).transpose(0, 2, 1))
    q01_til = np.ascontiguousarray(
        q01.reshape(B, Lq // 128, 128).transpose(0, 2, 1))

    in_maps = []
    for c in range(N_CORES):
        s = flat[c * BPC:(c + 1) * BPC]
        in_maps.append({
            "qT": np.ascontiguousarray(qT[s]),
            "qT16": np.ascontiguousarray(qT16[s]),
            "cT": np.ascontiguousarray(cT[s]),
            "cn": np.ascontiguousarray(cn[s]),
            "win": win, "wo1": wo1, "wf": wf,
            "kmin": np.ascontiguousarray(kmin_rep[s]),
            "qmin": np.ascontiguousarray(qmin_til[s]),
            "q01": np.ascontiguousarray(q01_til[s]),
            "m01": np.ascontiguousarray(m01[s]),
            "mean": np.ascontiguousarray(mean_c[s]),
            "cb": np.ascontiguousarray(cb[s]),
        })

    res = bass_utils.run_bass_kernel_spmd(nc, in_maps, core_ids=list(range(N_CORES)))
    _cache["last_result"] = res

    out = np.empty((B, Lq, D), dtype=np.float32)
    scores = np.empty((B, Lq, Lk), dtype=np.float32)
    for c in range(N_CORES):
        s = flat[c * BPC:(c + 1) * BPC]
        out[s] = res.results[c]["out"]
        scores[s] = res.results[c]["sc"]
    return out, scores


_program_cache = _cache  # test.py compatibility


# revision 19
# speedup vs baseline: 1.4413x; 1.0063x over previous
"""Trainium2 Bass kernel for masked general attention (ragged sequences).

reference computation per batch b:
    q       = query[b] @ W_in.T                      [Lq, D]
    S       = q @ context[b].T                       [Lq, Lk]
    S_m     = where(qmask & kmask, S, -1e9)
    W       = softmax(S_m, axis=-1)
    mix     = W @ context[b]                         [Lq, D]
    out     = tanh(concat([mix, q]) @ W_out.T)       [Lq, D]
    returns (out, S_m)

Sharding / specialization strategy:
- Data-parallel over batch: 32 batches / 8 cores, SPMD (one program).
- W_in is folded away on the host: scores = query @ (context @ W_in)^T
  via an on-device per-batch projection of the context (cw), and the
  query half of the output matmul uses Wfused = W_in.T @ W_out[:,D:].T
  so the per-block q-projection disappears entirely.
- Ragged-length specialization: batches are assigned to 4 "slots" (one
  batch per slot per core) minimizing the baked cost; per slot the
  program only computes attention for q-tiles below the slot's max
  query length and k-tiles below the slot's max context length.
  Skipped score regions are filled with the exact -1e9 constant; rows
  of fully-masked queries get uniform-softmax semantics via a rank-1
  (context-mean x masked-q-indicator) correction added into the mix
  accumulation, matching the reference bit-for-bit in structure.

dtypes: scores chain in float32r (TF32-class), mix/out matmuls in fp16,
softmax stats in fp32. Masking uses elementwise min against +BIG/-1e9
vectors, which reproduces the reference's exact -1e9 fill.
"""

import sys

sys.path.insert(0, "/opt/trn_rl_repo")

import random

import numpy as np
import ml_dtypes

import concourse.bass as bass
import concourse.tile as tile
from concourse import bacc, mybir
from concourse import bass_utils
from concourse.masks import make_identity

F32 = mybir.dt.float32
F32R = mybir.dt.float32r
FP16 = mybir.dt.float16
BF16 = mybir.dt.bfloat16

B, Lq, Lk, D = 32, 1024, 1024, 1024
N_CORES = 8
BPC = B // N_CORES          # batches (slots) per core
MQ = 256                    # queries per block
NBLK = Lq // MQ             # q-blocks per batch
NEG = -1e9
BIG = 3.0e38

_cache = {}


def _slot_cost(nq_max, nk_max):
    nqb = (nq_max + 1) // 2
    nkn = (nk_max + 3) // 4
    cw = nkn * 32768
    comp = nqb * (nkn * 8192 + nk_max * (256 + 2048) + 32768)
    skip = (4 - nqb) * 16384
    return cw + comp + skip


def _assign_slots(query_lengths, context_lengths):
    """Partition the 32 batches into 4 slots x 8 cores minimizing the
    baked per-slot cost. Returns perm[slot][core] -> batch index and the
    per-slot (NQB, NKN, NKT)."""
    nqt = -(-query_lengths.astype(np.int64) // 128)
    nkt = -(-context_lengths.astype(np.int64) // 128)
    order = np.argsort(nqt * nkt)
    slots = [list(order[j * N_CORES:(j + 1) * N_CORES]) for j in range(BPC)]

    def total(ss):
        return sum(_slot_cost(max(nqt[i] for i in s), max(nkt[i] for i in s))
                   for s in ss)

    best = total(slots)
    rng = random.Random(0)
    for _ in range(60000):
        a, b = rng.randrange(BPC), rng.randrange(BPC)
        if a == b:
            continue
        i, j = rng.randrange(N_CORES), rng.randrange(N_CORES)
        slots[a][i], slots[b][j] = slots[b][j], slots[a][i]
        c = total(slots)
        if c <= best:
            best = c
        else:
            slots[a][i], slots[b][j] = slots[b][j], slots[a][i]

    keyed = []
    for s in slots:
        nq, nk = max(nqt[i] for i in s), max(nkt[i] for i in s)
        keyed.append(((int((nk + 3) // 4), int(nk), int((nq + 1) // 2)), s))
    keyed.sort(key=lambda kv: kv[0])
    slots = [s for _, s in keyed]
    params = [(k[2], k[0], k[1]) for k, _ in keyed]
    return slots, tuple(params)


def _build_program(params):
    """params: tuple of (NQB, NKN, NKT) per slot."""
    nc = bacc.Bacc("TRN2", target_bir_lowering=False, debug=False,
                   num_devices=N_CORES)

    qT_d = nc.dram_tensor("qT", [BPC, D, Lq], F32R, kind="ExternalInput").ap()
    qT16_d = nc.dram_tensor("qT16", [BPC, D, Lq], FP16, kind="ExternalInput").ap()
    cT_d = nc.dram_tensor("cT", [BPC, D, Lk], F32R, kind="ExternalInput").ap()
    cn_d = nc.dram_tensor("cn", [BPC, Lk, D], FP16, kind="ExternalInput").ap()
    win_d = nc.dram_tensor("win", [D, D], F32R, kind="ExternalInput").ap()
    wo1_d = nc.dram_tensor("wo1", [D, D], FP16, kind="ExternalInput").ap()
    wf_d = nc.dram_tensor("wf", [D, D], FP16, kind="ExternalInput").ap()
    kmin_d = nc.dram_tensor("kmin", [BPC, 128, Lk], F32, kind="ExternalInput").ap()
    qmin_d = nc.dram_tensor("qmin", [BPC, 128, Lq // 128], F32, kind="ExternalInput").ap()
    q01_d = nc.dram_tensor("q01", [BPC, 128, Lq // 128], F32, kind="ExternalInput").ap()
    m01_d = nc.dram_tensor("m01", [BPC, Lq], FP16, kind="ExternalInput").ap()
    mean_d = nc.dram_tensor("mean", [BPC, D], FP16, kind="ExternalInput").ap()
    cb_d = nc.dram_tensor("cb", [BPC, D], FP16, kind="ExternalInput").ap()

    out_d = nc.dram_tensor("out", [BPC, Lq, D], F32, kind="ExternalOutput").ap()
    sc_d = nc.dram_tensor("sc", [BPC, Lq, Lk], F32, kind="ExternalOutput").ap()

    with tile.TileContext(nc) as tc:
        with (
            tc.tile_pool(name="static", bufs=1) as st,
            tc.tile_pool(name="ctx", bufs=1) as ctx_pool,
            tc.tile_pool(name="qry", bufs=2) as qry_pool,
            tc.tile_pool(name="q16", bufs=1) as q16_pool,
            tc.tile_pool(name="q16s", bufs=1) as q16s_pool,
            tc.tile_pool(name="ew", bufs=2) as ew_pool,
            tc.tile_pool(name="wm", bufs=2) as wm_pool,
            tc.tile_pool(name="sm", bufs=2) as sm_pool,
            tc.tile_pool(name="ot", bufs=2) as ot_pool,
            tc.tile_pool(name="stats", bufs=4) as stats_pool,
            tc.tile_pool(name="psA", bufs=2, space="PSUM") as psA,
            tc.tile_pool(name="psS", bufs=2, space="PSUM") as psS,
            tc.tile_pool(name="psO", bufs=2, space="PSUM") as psO,
            tc.tile_pool(name="psT", bufs=2, space="PSUM") as psT,
        ):
            win_sb = st.tile([128, 8 * D], F32R, tag="win")
            for et in range(8):
                nc.sync.dma_start(win_sb[:, et * D:(et + 1) * D],
                                  win_d[et * 128:(et + 1) * 128, :])

            def qry_dma(b, blk_i):
                q0 = blk_i * MQ
                t = qry_pool.tile([128, 8 * MQ], F32R, tag="qry")
                for dt in range(8):
                    nc.sync.dma_start(
                        t[:, dt * MQ:(dt + 1) * MQ],
                        qT_d[b, dt * 128:(dt + 1) * 128, q0:q0 + MQ])
                return t

            def qry16_dma(b, blk_i, pool=None, tag="q16"):
                q0 = blk_i * MQ
                pool = pool or q16_pool
                t = pool.tile([128, 8 * MQ], FP16, tag=tag)
                for dt in range(8):
                    nc.sync.dma_start(
                        t[:, dt * MQ:(dt + 1) * MQ],
                        qT16_d[b, dt * 128:(dt + 1) * 128, q0:q0 + MQ])
                return t

            def load_cT(b):
                """Scores-side context; its slot frees at the previous
                batch's cw build, so it can prefetch a block early."""
                NQB, NKN, NKT = params[b]
                cT_sb = ctx_pool.tile([128, 8 * Lk], F32R, tag="cT")
                for et in range(8):
                    nc.sync.dma_start(
                        cT_sb[:, et * Lk:et * Lk + NKN * 512],
                        cT_d[b, et * 128:(et + 1) * 128, :NKN * 512])
                return dict(cT=cT_sb)

            def load_masks(b, ctx):
                """Mask tiles: slots free at the previous batch's last
                scores chain, so these load at the crossing itself."""
                NQB, NKN, NKT = params[b]
                kmin_sb = ctx_pool.tile([128, Lk], F32, tag="kmin")
                nc.sync.dma_start(kmin_sb[:, :NKN * 512], kmin_d[b, :, :NKN * 512])
                qmin_sb = ctx_pool.tile([128, Lq // 128], F32, tag="qmin")
                nc.sync.dma_start(qmin_sb[:], qmin_d[b])
                q01_sb = ctx_pool.tile([128, Lq // 128], F32, tag="q01")
                nc.sync.dma_start(q01_sb[:], q01_d[b])
                ctx.update(kmin=kmin_sb, qmin=qmin_sb, q01=q01_sb)

            def load_ctx_late(b, ctx):
                """Tiles whose slots are released only by the previous
                batch's last mix: emitted after that mix to avoid blocking
                the in-order DMA queue behind an unsatisfiable wait."""
                NQB, NKN, NKT = params[b]
                cn_sb = ctx_pool.tile([128, 8 * D], FP16, tag="cn")
                for kt in range(NKT):
                    nc.sync.dma_start(cn_sb[:, kt * D:(kt + 1) * D],
                                      cn_d[b, kt * 128:(kt + 1) * 128, :])
                m01_sb = ctx_pool.tile([1, Lq], FP16, tag="m01")
                nc.sync.dma_start(m01_sb[:], m01_d[b:b + 1, :])
                mean_sb = ctx_pool.tile([1, D], FP16, tag="mean")
                nc.sync.dma_start(mean_sb[:], mean_d[b:b + 1, :])
                cb_sb = ctx_pool.tile([1, D], FP16, tag="cb")
                nc.sync.dma_start(cb_sb[:], cb_d[b:b + 1, :])
                ctx.update(cn=cn_sb, m01=m01_sb, mean=mean_sb, cb=cb_sb)

            def cw_build(b, ctx):
                """cw[d, k] = sum_e W_in[e, d] * contextT[e, k] (f32r)."""
                NQB, NKN, NKT = params[b]
                cw_sb = ctx_pool.tile([128, 8 * Lk], F32R, tag="cw")
                for dt in range(8):
                    for n in range(NKN):
                        ps = psS.tile([128, 512], F32, tag="psS")
                        for et in range(8):
                            nc.tensor.matmul(
                                ps[:],
                                win_sb[:, et * D + dt * 128:et * D + (dt + 1) * 128],
                                ctx["cT"][:, et * Lk + n * 512:et * Lk + (n + 1) * 512],
                                start=(et == 0), stop=(et == 7))
                        nc.vector.tensor_copy(
                            cw_sb[:, dt * Lk + n * 512:dt * Lk + (n + 1) * 512],
                            ps[:])
                ctx["cw"] = cw_sb

            # one [128, 1024] tile of exact -1e9 for skipped score regions,
            # plus a [1, 128] ones row for the constant-output rank-1 matmul
            const_sb = st.tile([128, 512], F32, tag="const")
            nc.vector.memset(const_sb[:], NEG)
            ones_sb = st.tile([1, 128], FP16, tag="ones")
            nc.vector.memset(ones_sb[:], 1.0)

            def scores_softmax(b, blk_i, qry_sb, ctx):
                """Masked scores -> DRAM; softmax weights -> ew tile."""
                NQB, NKN, NKT = params[b]
                q0 = blk_i * MQ
                ew_sb = ew_pool.tile([128, 2 * Lk], FP16, tag="ew")
                for h in range(2):
                    jt = blk_i * 2 + h
                    rows = slice(q0 + h * 128, q0 + (h + 1) * 128)
                    stt = stats_pool.tile([128, 8], F32, tag="stats")
                    sm_n = []
                    for n in range(NKN):
                        ps = psS.tile([128, 512], F32, tag="psS")
                        for dt in range(8):
                            nc.tensor.matmul(
                                ps[:],
                                qry_sb[:, dt * MQ + h * 128:dt * MQ + (h + 1) * 128],
                                ctx["cw"][:, dt * Lk + n * 512:dt * Lk + (n + 1) * 512],
                                start=(dt == 0), stop=(dt == 7))
                        sm = sm_pool.tile([128, 512], F32, tag="sm")
                        sm_n.append(sm)
                        nc.vector.tensor_tensor(
                            sm[:], ps[:], ctx["kmin"][:, n * 512:(n + 1) * 512],
                            op=mybir.AluOpType.min)
                        nc.vector.tensor_scalar_min(
                            sm[:], sm[:], ctx["qmin"][:, jt:jt + 1])
                        nc.sync.dma_start(sc_d[b, rows, n * 512:(n + 1) * 512],
                                          sm[:])
                        nc.vector.reduce_max(
                            stt[:, n:n + 1], sm[:],
                            axis=mybir.AxisListType.X, negate=True)
                    if NKN == 1:
                        nc.sync.dma_start(sc_d[b, rows, 512:], const_sb[:])
                        negm = stt[:, 0:1]
                    else:
                        nc.vector.tensor_tensor(
                            stt[:, 2:3], stt[:, 0:1], stt[:, 1:2],
                            op=mybir.AluOpType.min)
                        negm = stt[:, 2:3]
                    for n in range(NKN):
                        nc.scalar.activation(
                            ew_sb[:, h * Lk + n * 512:h * Lk + (n + 1) * 512],
                            sm_n[n][:],
                            mybir.ActivationFunctionType.Exp,
                            bias=negm, scale=1.0,
                            accum_out=stt[:, 3 + n:4 + n])
                    if NKN == 1:
                        ssum = stt[:, 3:4]
                    else:
                        nc.vector.tensor_tensor(
                            stt[:, 5:6], stt[:, 3:4], stt[:, 4:5],
                            op=mybir.AluOpType.add)
                        ssum = stt[:, 5:6]
                    nc.vector.reciprocal(stt[:, 6:7], ssum)
                    if NKT == 8:
                        scale = stt[:, 6:7]
                    else:
                        # zero the weights of fully-masked query rows; their
                        # uniform mix is re-added as a rank-1 term in stage 3
                        nc.vector.tensor_tensor(
                            stt[:, 7:8], stt[:, 6:7], ctx["q01"][:, jt:jt + 1],
                            op=mybir.AluOpType.mult)
                        scale = stt[:, 7:8]
                    nc.vector.tensor_scalar_mul(
                        ew_sb[:, h * Lk:h * Lk + NKN * 512],
                        ew_sb[:, h * Lk:h * Lk + NKN * 512],
                        scale)
                return ew_sb

            def transposes(b, ew_sb, ident):
                NQB, NKN, NKT = params[b]
                wt_sb = wm_pool.tile([128, 8 * MQ], FP16, tag="wm")
                for kt in range(NKT):
                    pt = psT.tile([128, MQ], FP16, tag="psT")
                    for h in range(2):
                        nc.tensor.transpose(
                            pt[:, h * 128:(h + 1) * 128],
                            ew_sb[:, h * Lk + kt * 128:h * Lk + (kt + 1) * 128],
                            ident[:])
                    nc.vector.tensor_copy(wt_sb[:, kt * MQ:(kt + 1) * MQ], pt[:])
                return wt_sb

            def mix_stage(b, blk_i, wt_sb, ctx):
                NQB, NKN, NKT = params[b]
                q0 = blk_i * MQ
                mixT_sb = wm_pool.tile([128, 8 * MQ], FP16, tag="wm")
                for dt in range(8):
                    pm = psA.tile([128, MQ], F32, tag="psA")
                    for kt in range(NKT):
                        nc.tensor.matmul(
                            pm[:],
                            ctx["cn"][:, kt * D + dt * 128:kt * D + (dt + 1) * 128],
                            wt_sb[:, kt * MQ:(kt + 1) * MQ],
                            start=(kt == 0), stop=(kt == NKT - 1 and NKT == 8))
                        # rank-1: uniform context-mean for fully-masked queries
                    if NKT < 8:
                        nc.tensor.matmul(
                            pm[:],
                            ctx["mean"][0:1, dt * 128:(dt + 1) * 128],
                            ctx["m01"][0:1, q0:q0 + MQ],
                            start=False, stop=True)
                    nc.vector.tensor_copy(mixT_sb[:, dt * MQ:(dt + 1) * MQ], pm[:])
                return mixT_sb

            def out_stage(b, blk_i, q16_sb, mixT_sb):
                q0 = blk_i * MQ
                for h in range(2):
                    rows = slice(q0 + h * 128, q0 + (h + 1) * 128)
                    for n in range(2):
                        po = psO.tile([128, 512], F32, tag="psO")
                        for dt in range(8):
                            nc.tensor.matmul(
                                po[:],
                                mixT_sb[:, dt * MQ + h * 128:dt * MQ + (h + 1) * 128],
                                wo1_sb[:, dt * D + n * 512:dt * D + (n + 1) * 512],
                                start=(dt == 0), stop=False)
                        for dt in range(8):
                            nc.tensor.matmul(
                                po[:],
                                q16_sb[:, dt * MQ + h * 128:dt * MQ + (h + 1) * 128],
                                wf_sb[:, dt * D + n * 512:dt * D + (n + 1) * 512],
                                start=False, stop=(dt == 7))
                        ot = ot_pool.tile([128, 512], F32, tag="ot")
                        nc.scalar.activation(
                            ot[:], po[:], mybir.ActivationFunctionType.Tanh)
                        nc.sync.dma_start(out_d[b, rows, n * 512:(n + 1) * 512],
                                          ot[:])

            def skipped_block(b, blk_i, ctx):
                """q-block past every query length in the slot: scores are
                all -1e9; out = tanh(query@Wfused + mean_ctx@Wo1)."""
                q0 = blk_i * MQ
                q16_sb = qry16_dma(b, blk_i, pool=q16s_pool, tag="q16s")
                for h in range(2):
                    rows = slice(q0 + h * 128, q0 + (h + 1) * 128)
                    for n4 in range(2):
                        nc.sync.dma_start(
                            sc_d[b, rows, n4 * 512:(n4 + 1) * 512], const_sb[:])
                    for n in range(2):
                        po = psO.tile([128, 512], F32, tag="psO")
                        for dt in range(8):
                            nc.tensor.matmul(
                                po[:],
                                q16_sb[:, dt * MQ + h * 128:dt * MQ + (h + 1) * 128],
                                wf_sb[:, dt * D + n * 512:dt * D + (n + 1) * 512],
                                start=(dt == 0), stop=False)
                        nc.tensor.matmul(
                            po[:], ones_sb[0:1, :],
                            ctx["cb"][0:1, n * 512:(n + 1) * 512],
                            start=False, stop=True)
                        ot = ot_pool.tile([128, 512], F32, tag="ot")
                        nc.scalar.activation(
                            ot[:], po[:], mybir.ActivationFunctionType.Tanh)
                        nc.sync.dma_start(out_d[b, rows, n * 512:(n + 1) * 512],
                                          ot[:])

            # ---- prologue ---------------------------------------------
            qry0 = qry_dma(0, 0)
            ctx0 = load_cT(0)
            load_masks(0, ctx0)
            load_ctx_late(0, ctx0)

            wo1_sb = st.tile([128, 8 * D], FP16, tag="wo1")
            for ct in range(8):
                nc.sync.dma_start(wo1_sb[:, ct * D:(ct + 1) * D],
                                  wo1_d[ct * 128:(ct + 1) * 128, :])
            wf_sb = st.tile([128, 8 * D], FP16, tag="wf")
            for ct in range(8):
                nc.sync.dma_start(wf_sb[:, ct * D:(ct + 1) * D],
                                  wf_d[ct * 128:(ct + 1) * 128, :])
            ident = st.tile([128, 128], FP16, tag="ident")
            make_identity(nc, ident[:])

            cw_build(0, ctx0)

            # flattened computed-block sequence with one-block lookahead:
            # scores(next) is emitted before transposes(cur) so the PE has
            # work while the softmax chain runs on DVE/ACT.
            seq = [(b, i) for b in range(BPC) for i in range(params[b][0])]
            cur_ctx = {0: ctx0}
            q16_0 = qry16_dma(0, 0)
            pend = (0, 0, q16_0, scores_softmax(0, 0, qry0, ctx0))
            for idx in range(len(seq)):
                b, i = seq[idx]
                _, _, q16_sb, ew_sb = pend
                ctx = cur_ctx[b]
                # prefetch the next batch's scores-side context one block
                # before the crossing so the cw build never waits on DMA
                if idx + 2 < len(seq) and seq[idx + 2][0] != seq[idx + 1][0]:
                    fb = seq[idx + 2][0]
                    cur_ctx[fb] = load_cT(fb)
                nxt = seq[idx + 1] if idx + 1 < len(seq) else None
                if nxt is not None:
                    nb, ni = nxt
                    if nb != b:
                        if nb not in cur_ctx:
                            cur_ctx[nb] = load_cT(nb)
                        load_masks(nb, cur_ctx[nb])
                        cw_build(nb, cur_ctx[nb])
                    nqry = qry_dma(nb, ni)
                    sew = scores_softmax(nb, ni, nqry, cur_ctx[nb])
                    pend = (nb, ni, qry16_dma(nb, ni), sew)
                wt_sb = transposes(b, ew_sb, ident)
                mixT_sb = mix_stage(b, i, wt_sb, ctx)
                out_stage(b, i, q16_sb, mixT_sb)
                if nxt is None or nxt[0] != b:
                    for si in range(params[b][0], NBLK):
                        skipped_block(b, si, ctx)
                if nxt is not None and nxt[0] != b:
                    load_ctx_late(nxt[0], cur_ctx[nxt[0]])

    nc.compile()
    return nc


def kernel(query, context, query_lengths, context_lengths, W_in, W_out):
    slots, params = _assign_slots(np.asarray(query_lengths),
                                  np.asarray(context_lengths))
    if _cache.get("params") != params:
        _cache["nc"] = _build_program(params)
        _cache["params"] = params
    nc = _cache["nc"]

    # batch order: core c processes batches [slots[0][c], slots[1][c], ...]
    perm = np.array(slots)                       # [BPC, N_CORES]
    flat = perm.T.reshape(-1)                    # core-major batch order

    query = np.asarray(query, dtype=np.float32)
    context = np.asarray(context, dtype=np.float32)
    ql = np.asarray(query_lengths).astype(np.int64)
    cl = np.asarray(context_lengths).astype(np.int64)

    qT = np.ascontiguousarray(query.transpose(0, 2, 1))
    qT16 = qT.astype(np.float16)
    cT = np.ascontiguousarray(context.transpose(0, 2, 1))
    cn = context.astype(np.float16)
    win = np.ascontiguousarray(W_in, dtype=np.float32)
    woT = np.ascontiguousarray(W_out.T, dtype=np.float32)
    wo1 = woT[:D].astype(np.float16)
    wf = (W_in.astype(np.float64).T @ woT[D:].astype(np.float64)).astype(np.float16)
    mean_c = context.astype(np.float64).mean(axis=1)           # [B, D]
    cb = (mean_c @ woT[:D].astype(np.float64)).astype(np.float16)
    mean_c = mean_c.astype(np.float16)

    k_idx = np.arange(Lk)
    q_idx = np.arange(Lq)
    kvalid = k_idx[None, :] < cl[:, None]
    qvalid = q_idx[None, :] < ql[:, None]
    kmin = np.where(kvalid, np.float32(BIG), np.float32(NEG)).astype(np.float32)
    qmin = np.where(qvalid, np.float32(BIG), np.float32(NEG)).astype(np.float32)
    q01 = qvalid.astype(np.float32)
    m01 = (~qvalid).astype(np.float16)
    kmin_rep = np.ascontiguousarray(
        np.broadcast_to(kmin[:, None, :], (B, 128, Lk)))
    qmin_til = np.ascontiguousarray(
        qmin.reshape(B, Lq // 128, 128).transpose(0, 2, 1))
    q01_til = np.ascontiguousarray(
        q01.reshape(B, Lq // 128, 128).transpose(0, 2, 1))

    in_maps = []
    for c in range(N_CORES):
        s = flat[c * BPC:(c + 1) * BPC]
        in_maps.append({
            "qT": np.ascontiguousarray(qT[s]),
            "qT16": np.ascontiguousarray(qT16[s]),
            "cT": np.ascontiguousarray(cT[s]),
            "cn": np.ascontiguousarray(cn[s]),
            "win": win, "wo1": wo1, "wf": wf,
            "kmin": np.ascontiguousarray(kmin_rep[s]),
            "qmin": np.ascontiguousarray(qmin_til[s]),
            "q01": np.ascontiguousarray(q01_til[s]),
            "m01": np.ascontiguousarray(m01[s]),
            "mean": np.ascontiguousarray(mean_c[s]),
            "cb": np.ascontiguousarray(cb[s]),
        })

    res = bass_utils.run_bass_kernel_spmd(nc, in_maps, core_ids=list(range(N_CORES)))
    _cache["last_result"] = res

    out = np.empty((B, Lq, D), dtype=np.float32)
    scores = np.empty((B, Lq, Lk), dtype=np.float32)
    for c in range(N_CORES):
        s = flat[c * BPC:(c + 1) * BPC]
        out[s] = res.results[c]["out"]
        scores[s] = res.results[c]["sc"]
    return out, scores


_program_cache = _cache  # test.py compatibility


# revision 20
# speedup vs baseline: 1.4730x; 1.0220x over previous
"""Trainium2 Bass kernel for masked general attention (ragged sequences).

reference computation per batch b:
    q       = query[b] @ W_in.T                      [Lq, D]
    S       = q @ context[b].T                       [Lq, Lk]
    S_m     = where(qmask & kmask, S, -1e9)
    W       = softmax(S_m, axis=-1)
    mix     = W @ context[b]                         [Lq, D]
    out     = tanh(concat([mix, q]) @ W_out.T)       [Lq, D]
    returns (out, S_m)

Sharding / specialization strategy:
- Data-parallel over batch: 32 batches / 8 cores, SPMD (one program).
- W_in is folded away on the host: scores = query @ (context @ W_in)^T
  via an on-device per-batch projection of the context (cw), and the
  query half of the output matmul uses Wfused = W_in.T @ W_out[:,D:].T
  so the per-block q-projection disappears entirely.
- Ragged-length specialization: batches are assigned to 4 "slots" (one
  batch per slot per core) minimizing the baked cost; per slot the
  program only computes attention for q-tiles below the slot's max
  query length and k-tiles below the slot's max context length.
  Skipped score regions are filled with the exact -1e9 constant; rows
  of fully-masked queries get uniform-softmax semantics via a rank-1
  (context-mean x masked-q-indicator) correction added into the mix
  accumulation, matching the reference bit-for-bit in structure.

dtypes: scores chain in float32r (TF32-class), mix/out matmuls in fp16,
softmax stats in fp32. Masking uses elementwise min against +BIG/-1e9
vectors, which reproduces the reference's exact -1e9 fill.
"""

import sys

sys.path.insert(0, "/opt/trn_rl_repo")

import random

import numpy as np
import ml_dtypes

import concourse.bass as bass
import concourse.tile as tile
from concourse import bacc, mybir
from concourse import bass_utils
from concourse.masks import make_identity

F32 = mybir.dt.float32
F32R = mybir.dt.float32r
FP16 = mybir.dt.float16
BF16 = mybir.dt.bfloat16

B, Lq, Lk, D = 32, 1024, 1024, 1024
N_CORES = 8
BPC = B // N_CORES          # batches (slots) per core
MQ = 256                    # queries per block
NBLK = Lq // MQ             # q-blocks per batch
NEG = -1e9
BIG = 3.0e38

_cache = {}


def _k_chunks(nkt):
    """Score-column chunk widths: each in [256, 512] (fp32r full rate),
    covering nkt*128 columns with minimal padding."""
    total = max(256, nkt * 128)
    if total <= 512:
        return [total]
    if total <= 768:
        return [total - 256, 256]
    return [512, total - 512]


def _slot_cost(nq_max, nk_max):
    nqb = (nq_max + 1) // 2
    S = sum(_k_chunks(nk_max))
    cw = 64 * S
    comp = nqb * (16 * S + nk_max * (256 + 2048) + 32768)
    skip = (4 - nqb) * 16384
    return cw + comp + skip


def _assign_slots(query_lengths, context_lengths):
    """Partition the 32 batches into 4 slots x 8 cores minimizing the
    baked per-slot cost. Returns perm[slot][core] -> batch index and the
    per-slot (NQB, NKN, NKT)."""
    nqt = -(-query_lengths.astype(np.int64) // 128)
    nkt = -(-context_lengths.astype(np.int64) // 128)
    order = np.argsort(nqt * nkt)
    slots = [list(order[j * N_CORES:(j + 1) * N_CORES]) for j in range(BPC)]

    def total(ss):
        return sum(_slot_cost(max(nqt[i] for i in s), max(nkt[i] for i in s))
                   for s in ss)

    best = total(slots)
    rng = random.Random(0)
    for _ in range(60000):
        a, b = rng.randrange(BPC), rng.randrange(BPC)
        if a == b:
            continue
        i, j = rng.randrange(N_CORES), rng.randrange(N_CORES)
        slots[a][i], slots[b][j] = slots[b][j], slots[a][i]
        c = total(slots)
        if c <= best:
            best = c
        else:
            slots[a][i], slots[b][j] = slots[b][j], slots[a][i]

    keyed = []
    for s in slots:
        nq, nk = max(nqt[i] for i in s), max(nkt[i] for i in s)
        keyed.append(((int((nk + 3) // 4), int(nk), int((nq + 1) // 2)), s))
    keyed.sort(key=lambda kv: kv[0])
    slots = [s for _, s in keyed]
    params = [(k[2], k[0], k[1]) for k, _ in keyed]
    return slots, tuple(params)


def _build_program(params):
    """params: tuple of (NQB, NKN, NKT) per slot."""
    nc = bacc.Bacc("TRN2", target_bir_lowering=False, debug=False,
                   num_devices=N_CORES)

    qT_d = nc.dram_tensor("qT", [BPC, D, Lq], F32R, kind="ExternalInput").ap()
    qT16_d = nc.dram_tensor("qT16", [BPC, D, Lq], FP16, kind="ExternalInput").ap()
    cT_d = nc.dram_tensor("cT", [BPC, D, Lk], F32R, kind="ExternalInput").ap()
    cn_d = nc.dram_tensor("cn", [BPC, Lk, D], FP16, kind="ExternalInput").ap()
    win_d = nc.dram_tensor("win", [D, D], F32R, kind="ExternalInput").ap()
    wo1_d = nc.dram_tensor("wo1", [D, D], FP16, kind="ExternalInput").ap()
    wf_d = nc.dram_tensor("wf", [D, D], FP16, kind="ExternalInput").ap()
    kmin_d = nc.dram_tensor("kmin", [BPC, 128, Lk], F32, kind="ExternalInput").ap()
    qmin_d = nc.dram_tensor("qmin", [BPC, 128, Lq // 128], F32, kind="ExternalInput").ap()
    q01_d = nc.dram_tensor("q01", [BPC, 128, Lq // 128], F32, kind="ExternalInput").ap()
    m01_d = nc.dram_tensor("m01", [BPC, Lq], FP16, kind="ExternalInput").ap()
    mean_d = nc.dram_tensor("mean", [BPC, D], FP16, kind="ExternalInput").ap()
    cb_d = nc.dram_tensor("cb", [BPC, D], FP16, kind="ExternalInput").ap()

    out_d = nc.dram_tensor("out", [BPC, Lq, D], F32, kind="ExternalOutput").ap()
    sc_d = nc.dram_tensor("sc", [BPC, Lq, Lk], F32, kind="ExternalOutput").ap()

    with tile.TileContext(nc) as tc:
        with (
            tc.tile_pool(name="static", bufs=1) as st,
            tc.tile_pool(name="ctx", bufs=1) as ctx_pool,
            tc.tile_pool(name="qry", bufs=2) as qry_pool,
            tc.tile_pool(name="q16", bufs=1) as q16_pool,
            tc.tile_pool(name="q16s", bufs=1) as q16s_pool,
            tc.tile_pool(name="ew", bufs=2) as ew_pool,
            tc.tile_pool(name="wm", bufs=2) as wm_pool,
            tc.tile_pool(name="sm", bufs=2) as sm_pool,
            tc.tile_pool(name="ot", bufs=2) as ot_pool,
            tc.tile_pool(name="stats", bufs=4) as stats_pool,
            tc.tile_pool(name="psA", bufs=2, space="PSUM") as psA,
            tc.tile_pool(name="psS", bufs=2, space="PSUM") as psS,
            tc.tile_pool(name="psO", bufs=2, space="PSUM") as psO,
            tc.tile_pool(name="psT", bufs=2, space="PSUM") as psT,
        ):
            win_sb = st.tile([128, 8 * D], F32R, tag="win")
            for et in range(8):
                nc.sync.dma_start(win_sb[:, et * D:(et + 1) * D],
                                  win_d[et * 128:(et + 1) * 128, :])

            def qry_dma(b, blk_i):
                q0 = blk_i * MQ
                t = qry_pool.tile([128, 8 * MQ], F32R, tag="qry")
                for dt in range(8):
                    nc.sync.dma_start(
                        t[:, dt * MQ:(dt + 1) * MQ],
                        qT_d[b, dt * 128:(dt + 1) * 128, q0:q0 + MQ])
                return t

            def qry16_dma(b, blk_i, pool=None, tag="q16"):
                q0 = blk_i * MQ
                pool = pool or q16_pool
                t = pool.tile([128, 8 * MQ], FP16, tag=tag)
                for dt in range(8):
                    nc.sync.dma_start(
                        t[:, dt * MQ:(dt + 1) * MQ],
                        qT16_d[b, dt * 128:(dt + 1) * 128, q0:q0 + MQ])
                return t

            def load_cT(b):
                """Scores-side context; its slot frees at the previous
                batch's cw build, so it can prefetch a block early."""
                NQB, NKN, NKT = params[b]
                S = sum(_k_chunks(NKT))
                cT_sb = ctx_pool.tile([128, 8 * Lk], F32R, tag="cT")
                for et in range(8):
                    nc.sync.dma_start(
                        cT_sb[:, et * Lk:et * Lk + S],
                        cT_d[b, et * 128:(et + 1) * 128, :S])
                return dict(cT=cT_sb)

            def load_masks(b, ctx):
                """Mask tiles: slots free at the previous batch's last
                scores chain, so these load at the crossing itself."""
                NQB, NKN, NKT = params[b]
                S = sum(_k_chunks(NKT))
                kmin_sb = ctx_pool.tile([128, Lk], F32, tag="kmin")
                nc.sync.dma_start(kmin_sb[:, :S], kmin_d[b, :, :S])
                qmin_sb = ctx_pool.tile([128, Lq // 128], F32, tag="qmin")
                nc.sync.dma_start(qmin_sb[:], qmin_d[b])
                q01_sb = ctx_pool.tile([128, Lq // 128], F32, tag="q01")
                nc.sync.dma_start(q01_sb[:], q01_d[b])
                ctx.update(kmin=kmin_sb, qmin=qmin_sb, q01=q01_sb)

            def load_ctx_late(b, ctx):
                """Tiles whose slots are released only by the previous
                batch's last mix: emitted after that mix to avoid blocking
                the in-order DMA queue behind an unsatisfiable wait."""
                NQB, NKN, NKT = params[b]
                cn_sb = ctx_pool.tile([128, 8 * D], FP16, tag="cn")
                for kt in range(NKT):
                    nc.sync.dma_start(cn_sb[:, kt * D:(kt + 1) * D],
                                      cn_d[b, kt * 128:(kt + 1) * 128, :])
                m01_sb = ctx_pool.tile([1, Lq], FP16, tag="m01")
                nc.sync.dma_start(m01_sb[:], m01_d[b:b + 1, :])
                mean_sb = ctx_pool.tile([1, D], FP16, tag="mean")
                nc.sync.dma_start(mean_sb[:], mean_d[b:b + 1, :])
                cb_sb = ctx_pool.tile([1, D], FP16, tag="cb")
                nc.sync.dma_start(cb_sb[:], cb_d[b:b + 1, :])
                ctx.update(cn=cn_sb, m01=m01_sb, mean=mean_sb, cb=cb_sb)

            def cw_build(b, ctx):
                """cw[d, k] = sum_e W_in[e, d] * contextT[e, k] (f32r)."""
                NQB, NKN, NKT = params[b]
                cw_sb = ctx_pool.tile([128, 8 * Lk], F32R, tag="cw")
                for dt in range(8):
                    off = 0
                    for w in _k_chunks(NKT):
                        ps = psS.tile([128, 512], F32, tag="psS")
                        for et in range(8):
                            nc.tensor.matmul(
                                ps[:, :w],
                                win_sb[:, et * D + dt * 128:et * D + (dt + 1) * 128],
                                ctx["cT"][:, et * Lk + off:et * Lk + off + w],
                                start=(et == 0), stop=(et == 7))
                        nc.vector.tensor_copy(
                            cw_sb[:, dt * Lk + off:dt * Lk + off + w],
                            ps[:, :w])
                        off += w
                ctx["cw"] = cw_sb

            # one [128, 1024] tile of exact -1e9 for skipped score regions,
            # plus a [1, 128] ones row for the constant-output rank-1 matmul
            const_sb = st.tile([128, 512], F32, tag="const")
            nc.vector.memset(const_sb[:], NEG)
            ones_sb = st.tile([1, 128], FP16, tag="ones")
            nc.vector.memset(ones_sb[:], 1.0)

            def scores_softmax(b, blk_i, qry_sb, ctx):
                """Masked scores -> DRAM; softmax weights -> ew tile."""
                NQB, NKN, NKT = params[b]
                chunks = _k_chunks(NKT)
                S = sum(chunks)
                NCH = len(chunks)
                q0 = blk_i * MQ
                ew_sb = ew_pool.tile([128, 2 * Lk], FP16, tag="ew")
                for h in range(2):
                    jt = blk_i * 2 + h
                    rows = slice(q0 + h * 128, q0 + (h + 1) * 128)
                    stt = stats_pool.tile([128, 8], F32, tag="stats")
                    sm_n = []
                    off = 0
                    for n, w in enumerate(chunks):
                        ps = psS.tile([128, 512], F32, tag="psS")
                        for dt in range(8):
                            nc.tensor.matmul(
                                ps[:, :w],
                                qry_sb[:, dt * MQ + h * 128:dt * MQ + (h + 1) * 128],
                                ctx["cw"][:, dt * Lk + off:dt * Lk + off + w],
                                start=(dt == 0), stop=(dt == 7))
                        sm = sm_pool.tile([128, 512], F32, tag="sm")
                        sm_n.append((sm, off, w))
                        nc.vector.tensor_tensor(
                            sm[:, :w], ps[:, :w], ctx["kmin"][:, off:off + w],
                            op=mybir.AluOpType.min)
                        nc.vector.tensor_scalar_min(
                            sm[:, :w], sm[:, :w], ctx["qmin"][:, jt:jt + 1])
                        nc.sync.dma_start(sc_d[b, rows, off:off + w], sm[:, :w])
                        nc.vector.reduce_max(
                            stt[:, n:n + 1], sm[:, :w],
                            axis=mybir.AxisListType.X, negate=True)
                        off += w
                    # fill the uncomputed score columns with exact -1e9
                    coff = S
                    while coff < Lk:
                        cw_w = min(512, Lk - coff)
                        nc.sync.dma_start(sc_d[b, rows, coff:coff + cw_w],
                                          const_sb[:, :cw_w])
                        coff += cw_w
                    if NCH == 1:
                        negm = stt[:, 0:1]
                    else:
                        nc.vector.tensor_tensor(
                            stt[:, 2:3], stt[:, 0:1], stt[:, 1:2],
                            op=mybir.AluOpType.min)
                        negm = stt[:, 2:3]
                    for n, (sm, off, w) in enumerate(sm_n):
                        nc.scalar.activation(
                            ew_sb[:, h * Lk + off:h * Lk + off + w],
                            sm[:, :w],
                            mybir.ActivationFunctionType.Exp,
                            bias=negm, scale=1.0,
                            accum_out=stt[:, 3 + n:4 + n])
                    if NCH == 1:
                        ssum = stt[:, 3:4]
                    else:
                        nc.vector.tensor_tensor(
                            stt[:, 5:6], stt[:, 3:4], stt[:, 4:5],
                            op=mybir.AluOpType.add)
                        ssum = stt[:, 5:6]
                    nc.vector.reciprocal(stt[:, 6:7], ssum)
                    if NKT == 8:
                        scale = stt[:, 6:7]
                    else:
                        # zero the weights of fully-masked query rows; their
                        # uniform mix is re-added as a rank-1 term in stage 3
                        nc.vector.tensor_tensor(
                            stt[:, 7:8], stt[:, 6:7], ctx["q01"][:, jt:jt + 1],
                            op=mybir.AluOpType.mult)
                        scale = stt[:, 7:8]
                    nc.vector.tensor_scalar_mul(
                        ew_sb[:, h * Lk:h * Lk + S],
                        ew_sb[:, h * Lk:h * Lk + S],
                        scale)
                return ew_sb

            def transposes(b, ew_sb, ident):
                NQB, NKN, NKT = params[b]
                wt_sb = wm_pool.tile([128, 8 * MQ], FP16, tag="wm")
                for kt in range(NKT):
                    pt = psT.tile([128, MQ], FP16, tag="psT")
                    for h in range(2):
                        nc.tensor.transpose(
                            pt[:, h * 128:(h + 1) * 128],
                            ew_sb[:, h * Lk + kt * 128:h * Lk + (kt + 1) * 128],
                            ident[:])
                    nc.vector.tensor_copy(wt_sb[:, kt * MQ:(kt + 1) * MQ], pt[:])
                return wt_sb

            def mix_stage(b, blk_i, wt_sb, ctx):
                NQB, NKN, NKT = params[b]
                q0 = blk_i * MQ
                mixT_sb = wm_pool.tile([128, 8 * MQ], FP16, tag="wm")
                for dt in range(8):
                    pm = psA.tile([128, MQ], F32, tag="psA")
                    for kt in range(NKT):
                        nc.tensor.matmul(
                            pm[:],
                            ctx["cn"][:, kt * D + dt * 128:kt * D + (dt + 1) * 128],
                            wt_sb[:, kt * MQ:(kt + 1) * MQ],
                            start=(kt == 0), stop=(kt == NKT - 1 and NKT == 8))
                        # rank-1: uniform context-mean for fully-masked queries
                    if NKT < 8:
                        nc.tensor.matmul(
                            pm[:],
                            ctx["mean"][0:1, dt * 128:(dt + 1) * 128],
                            ctx["m01"][0:1, q0:q0 + MQ],
                            start=False, stop=True)
                    nc.vector.tensor_copy(mixT_sb[:, dt * MQ:(dt + 1) * MQ], pm[:])
                return mixT_sb

            def out_stage(b, blk_i, q16_sb, mixT_sb):
                q0 = blk_i * MQ
                for h in range(2):
                    rows = slice(q0 + h * 128, q0 + (h + 1) * 128)
                    for n in range(2):
                        po = psO.tile([128, 512], F32, tag="psO")
                        for dt in range(8):
                            nc.tensor.matmul(
                                po[:],
                                mixT_sb[:, dt * MQ + h * 128:dt * MQ + (h + 1) * 128],
                                wo1_sb[:, dt * D + n * 512:dt * D + (n + 1) * 512],
                                start=(dt == 0), stop=False)
                        for dt in range(8):
                            nc.tensor.matmul(
                                po[:],
                                q16_sb[:, dt * MQ + h * 128:dt * MQ + (h + 1) * 128],
                                wf_sb[:, dt * D + n * 512:dt * D + (n + 1) * 512],
                                start=False, stop=(dt == 7))
                        ot = ot_pool.tile([128, 512], F32, tag="ot")
                        nc.scalar.activation(
                            ot[:], po[:], mybir.ActivationFunctionType.Tanh)
                        nc.sync.dma_start(out_d[b, rows, n * 512:(n + 1) * 512],
                                          ot[:])

            def skipped_block(b, blk_i, ctx):
                """q-block past every query length in the slot: scores are
                all -1e9; out = tanh(query@Wfused + mean_ctx@Wo1)."""
                q0 = blk_i * MQ
                q16_sb = qry16_dma(b, blk_i, pool=q16s_pool, tag="q16s")
                for h in range(2):
                    rows = slice(q0 + h * 128, q0 + (h + 1) * 128)
                    for n4 in range(2):
                        nc.sync.dma_start(
                            sc_d[b, rows, n4 * 512:(n4 + 1) * 512], const_sb[:])
                    for n in range(2):
                        po = psO.tile([128, 512], F32, tag="psO")
                        for dt in range(8):
                            nc.tensor.matmul(
                                po[:],
                                q16_sb[:, dt * MQ + h * 128:dt * MQ + (h + 1) * 128],
                                wf_sb[:, dt * D + n * 512:dt * D + (n + 1) * 512],
                                start=(dt == 0), stop=False)
                        nc.tensor.matmul(
                            po[:], ones_sb[0:1, :],
                            ctx["cb"][0:1, n * 512:(n + 1) * 512],
                            start=False, stop=True)
                        ot = ot_pool.tile([128, 512], F32, tag="ot")
                        nc.scalar.activation(
                            ot[:], po[:], mybir.ActivationFunctionType.Tanh)
                        nc.sync.dma_start(out_d[b, rows, n * 512:(n + 1) * 512],
                                          ot[:])

            # ---- prologue ---------------------------------------------
            qry0 = qry_dma(0, 0)
            ctx0 = load_cT(0)
            load_masks(0, ctx0)
            load_ctx_late(0, ctx0)

            wo1_sb = st.tile([128, 8 * D], FP16, tag="wo1")
            for ct in range(8):
                nc.sync.dma_start(wo1_sb[:, ct * D:(ct + 1) * D],
                                  wo1_d[ct * 128:(ct + 1) * 128, :])
            wf_sb = st.tile([128, 8 * D], FP16, tag="wf")
            for ct in range(8):
                nc.sync.dma_start(wf_sb[:, ct * D:(ct + 1) * D],
                                  wf_d[ct * 128:(ct + 1) * 128, :])
            ident = st.tile([128, 128], FP16, tag="ident")
            make_identity(nc, ident[:])

            cw_build(0, ctx0)

            # flattened computed-block sequence with one-block lookahead:
            # scores(next) is emitted before transposes(cur) so the PE has
            # work while the softmax chain runs on DVE/ACT.
            seq = [(b, i) for b in range(BPC) for i in range(params[b][0])]
            cur_ctx = {0: ctx0}
            q16_0 = qry16_dma(0, 0)
            pend = (0, 0, q16_0, scores_softmax(0, 0, qry0, ctx0))
            for idx in range(len(seq)):
                b, i = seq[idx]
                _, _, q16_sb, ew_sb = pend
                ctx = cur_ctx[b]
                # prefetch the next batch's scores-side context one block
                # before the crossing so the cw build never waits on DMA
                if idx + 2 < len(seq) and seq[idx + 2][0] != seq[idx + 1][0]:
                    fb = seq[idx + 2][0]
                    cur_ctx[fb] = load_cT(fb)
                nxt = seq[idx + 1] if idx + 1 < len(seq) else None
                if nxt is not None:
                    nb, ni = nxt
                    if nb != b:
                        if nb not in cur_ctx:
                            cur_ctx[nb] = load_cT(nb)
                        load_masks(nb, cur_ctx[nb])
                        cw_build(nb, cur_ctx[nb])
                    nqry = qry_dma(nb, ni)
                    sew = scores_softmax(nb, ni, nqry, cur_ctx[nb])
                    pend = (nb, ni, qry16_dma(nb, ni), sew)
                wt_sb = transposes(b, ew_sb, ident)
                mixT_sb = mix_stage(b, i, wt_sb, ctx)
                out_stage(b, i, q16_sb, mixT_sb)
                if nxt is None or nxt[0] != b:
                    for si in range(params[b][0], NBLK):
                        skipped_block(b, si, ctx)
                if nxt is not None and nxt[0] != b:
                    load_ctx_late(nxt[0], cur_ctx[nxt[0]])

    nc.compile()
    return nc


def kernel(query, context, query_lengths, context_lengths, W_in, W_out):
    slots, params = _assign_slots(np.asarray(query_lengths),
                                  np.asarray(context_lengths))
    if _cache.get("params") != params:
        _cache["nc"] = _build_program(params)
        _cache["params"] = params
    nc = _cache["nc"]

    # batch order: core c processes batches [slots[0][c], slots[1][c], ...]
    perm = np.array(slots)                       # [BPC, N_CORES]
    flat = perm.T.reshape(-1)                    # core-major batch order

    query = np.asarray(query, dtype=np.float32)
    context = np.asarray(context, dtype=np.float32)
    ql = np.asarray(query_lengths).astype(np.int64)
    cl = np.asarray(context_lengths).astype(np.int64)

    qT = np.ascontiguousarray(query.transpose(0, 2, 1))
    qT16 = qT.astype(np.float16)
    cT = np.ascontiguousarray(context.transpose(0, 2, 1))
    cn = context.astype(np.float16)
    win = np.ascontiguousarray(W_in, dtype=np.float32)
    woT = np.ascontiguousarray(W_out.T, dtype=np.float32)
    wo1 = woT[:D].astype(np.float16)
    wf = (W_in.astype(np.float64).T @ woT[D:].astype(np.float64)).astype(np.float16)
    mean_c = context.astype(np.float64).mean(axis=1)           # [B, D]
    cb = (mean_c @ woT[:D].astype(np.float64)).astype(np.float16)
    mean_c = mean_c.astype(np.float16)

    k_idx = np.arange(Lk)
    q_idx = np.arange(Lq)
    kvalid = k_idx[None, :] < cl[:, None]
    qvalid = q_idx[None, :] < ql[:, None]
    kmin = np.where(kvalid, np.float32(BIG), np.float32(NEG)).astype(np.float32)
    qmin = np.where(qvalid, np.float32(BIG), np.float32(NEG)).astype(np.float32)
    q01 = qvalid.astype(np.float32)
    m01 = (~qvalid).astype(np.float16)
    kmin_rep = np.ascontiguousarray(
        np.broadcast_to(kmin[:, None, :], (B, 128, Lk)))
    qmin_til = np.ascontiguousarray(
        qmin.reshape(B, Lq // 128, 128).transpose(0, 2, 1))
    q01_til = np.ascontiguousarray(
        q01.reshape(B, Lq // 128, 128).transpose(0, 2, 1))

    in_maps = []
    for c in range(N_CORES):
        s = flat[c * BPC:(c + 1) * BPC]
        in_maps.append({
            "qT": np.ascontiguousarray(qT[s]),
            "qT16": np.ascontiguousarray(qT16[s]),
            "cT": np.ascontiguousarray(cT[s]),
            "cn": np.ascontiguousarray(cn[s]),
            "win": win, "wo1": wo1, "wf": wf,
            "kmin": np.ascontiguousarray(kmin_rep[s]),
            "qmin": np.ascontiguousarray(qmin_til[s]),
            "q01": np.ascontiguousarray(q01_til[s]),
            "m01": np.ascontiguousarray(m01[s]),
            "mean": np.ascontiguousarray(mean_c[s]),
            "cb": np.ascontiguousarray(cb[s]),
        })

    res = bass_utils.run_bass_kernel_spmd(nc, in_maps, core_ids=list(range(N_CORES)))
    _cache["last_result"] = res

    out = np.empty((B, Lq, D), dtype=np.float32)
    scores = np.empty((B, Lq, Lk), dtype=np.float32)
    for c in range(N_CORES):
        s = flat[c * BPC:(c + 1) * BPC]
        out[s] = res.results[c]["out"]
        scores[s] = res.results[c]["sc"]
    return out, scores


_program_cache = _cache  # test.py compatibility


# revision 21
# speedup vs baseline: 1.4746x; 1.0011x over previous
"""Trainium2 Bass kernel for masked general attention (ragged sequences).

reference computation per batch b:
    q       = query[b] @ W_in.T                      [Lq, D]
    S       = q @ context[b].T                       [Lq, Lk]
    S_m     = where(qmask & kmask, S, -1e9)
    W       = softmax(S_m, axis=-1)
    mix     = W @ context[b]                         [Lq, D]
    out     = tanh(concat([mix, q]) @ W_out.T)       [Lq, D]
    returns (out, S_m)

Sharding / specialization strategy:
- Data-parallel over batch: 32 batches / 8 cores, SPMD (one program).
- W_in is folded away on the host: scores = query @ (context @ W_in)^T
  via an on-device per-batch projection of the context (cw), and the
  query half of the output matmul uses Wfused = W_in.T @ W_out[:,D:].T
  so the per-block q-projection disappears entirely.
- Ragged-length specialization: batches are assigned to 4 "slots" (one
  batch per slot per core) minimizing the baked cost; per slot the
  program only computes attention for q-tiles below the slot's max
  query length and k-tiles below the slot's max context length.
  Skipped score regions are filled with the exact -1e9 constant; rows
  of fully-masked queries get uniform-softmax semantics via a rank-1
  (context-mean x masked-q-indicator) correction added into the mix
  accumulation, matching the reference bit-for-bit in structure.

dtypes: scores chain in float32r (TF32-class), mix/out matmuls in fp16,
softmax stats in fp32. Masking uses elementwise min against +BIG/-1e9
vectors, which reproduces the reference's exact -1e9 fill.
"""

import sys

sys.path.insert(0, "/opt/trn_rl_repo")

import random

import numpy as np
import ml_dtypes

import concourse.bass as bass
import concourse.tile as tile
from concourse import bacc, mybir
from concourse import bass_utils
from concourse.masks import make_identity

F32 = mybir.dt.float32
F32R = mybir.dt.float32r
FP16 = mybir.dt.float16
BF16 = mybir.dt.bfloat16

B, Lq, Lk, D = 32, 1024, 1024, 1024
N_CORES = 8
BPC = B // N_CORES          # batches (slots) per core
MQ = 256                    # queries per block
NBLK = Lq // MQ             # q-blocks per batch
NEG = -1e9
BIG = 3.0e38

_cache = {}


def _k_chunks(nkt):
    """Score-column chunk widths: each in [256, 512] (fp32r full rate),
    covering nkt*128 columns with minimal padding."""
    total = max(256, nkt * 128)
    if total <= 512:
        return [total]
    if total <= 768:
        return [total - 256, 256]
    return [512, total - 512]


def _slot_cost(nq_max, nk_max):
    nqb = (nq_max + 1) // 2
    S = sum(_k_chunks(nk_max))
    cw = 64 * S
    comp = nqb * (16 * S + nk_max * (256 + 2048) + 32768)
    skip = (4 - nqb) * 16384
    return cw + comp + skip


def _assign_slots(query_lengths, context_lengths):
    """Partition the 32 batches into 4 slots x 8 cores minimizing the
    baked per-slot cost. Returns perm[slot][core] -> batch index and the
    per-slot (NQB, NKN, NKT)."""
    nqt = -(-query_lengths.astype(np.int64) // 128)
    nkt = -(-context_lengths.astype(np.int64) // 128)
    order = np.argsort(nqt * nkt)
    slots = [list(order[j * N_CORES:(j + 1) * N_CORES]) for j in range(BPC)]

    def total(ss):
        return sum(_slot_cost(max(nqt[i] for i in s), max(nkt[i] for i in s))
                   for s in ss)

    best = total(slots)
    rng = random.Random(0)
    for _ in range(60000):
        a, b = rng.randrange(BPC), rng.randrange(BPC)
        if a == b:
            continue
        i, j = rng.randrange(N_CORES), rng.randrange(N_CORES)
        slots[a][i], slots[b][j] = slots[b][j], slots[a][i]
        c = total(slots)
        if c <= best:
            best = c
        else:
            slots[a][i], slots[b][j] = slots[b][j], slots[a][i]

    keyed = []
    for s in slots:
        nq, nk = max(nqt[i] for i in s), max(nkt[i] for i in s)
        keyed.append(((int((nk + 3) // 4), int(nk), int((nq + 1) // 2)), s))
    keyed.sort(key=lambda kv: kv[0])
    slots = [s for _, s in keyed]
    params = [(k[2], k[0], k[1]) for k, _ in keyed]
    return slots, tuple(params)


def _build_program(params):
    """params: tuple of (NQB, NKN, NKT) per slot."""
    nc = bacc.Bacc("TRN2", target_bir_lowering=False, debug=False,
                   num_devices=N_CORES)

    qT_d = nc.dram_tensor("qT", [BPC, D, Lq], F32R, kind="ExternalInput").ap()
    qT16_d = nc.dram_tensor("qT16", [BPC, D, Lq], FP16, kind="ExternalInput").ap()
    cT_d = nc.dram_tensor("cT", [BPC, D, Lk], F32R, kind="ExternalInput").ap()
    cn_d = nc.dram_tensor("cn", [BPC, Lk, D], FP16, kind="ExternalInput").ap()
    win_d = nc.dram_tensor("win", [D, D], F32R, kind="ExternalInput").ap()
    wo1_d = nc.dram_tensor("wo1", [D, D], FP16, kind="ExternalInput").ap()
    wf_d = nc.dram_tensor("wf", [D, D], FP16, kind="ExternalInput").ap()
    kmin_d = nc.dram_tensor("kmin", [BPC, 128, Lk], F32, kind="ExternalInput").ap()
    qmin_d = nc.dram_tensor("qmin", [BPC, 128, Lq // 128], F32, kind="ExternalInput").ap()
    q01_d = nc.dram_tensor("q01", [BPC, 128, Lq // 128], F32, kind="ExternalInput").ap()
    m01_d = nc.dram_tensor("m01", [BPC, Lq], FP16, kind="ExternalInput").ap()
    mean_d = nc.dram_tensor("mean", [BPC, D], FP16, kind="ExternalInput").ap()
    cb_d = nc.dram_tensor("cb", [BPC, D], FP16, kind="ExternalInput").ap()

    out_d = nc.dram_tensor("out", [BPC, Lq, D], F32, kind="ExternalOutput").ap()
    sc_d = nc.dram_tensor("sc", [BPC, Lq, Lk], F32, kind="ExternalOutput").ap()

    with tile.TileContext(nc) as tc:
        with (
            tc.tile_pool(name="static", bufs=1) as st,
            tc.tile_pool(name="ctx", bufs=1) as ctx_pool,
            tc.tile_pool(name="qry", bufs=2) as qry_pool,
            tc.tile_pool(name="q16", bufs=1) as q16_pool,
            tc.tile_pool(name="q16s", bufs=1) as q16s_pool,
            tc.tile_pool(name="ew", bufs=2) as ew_pool,
            tc.tile_pool(name="wm", bufs=2) as wm_pool,
            tc.tile_pool(name="sm", bufs=2) as sm_pool,
            tc.tile_pool(name="ot", bufs=2) as ot_pool,
            tc.tile_pool(name="stats", bufs=4) as stats_pool,
            tc.tile_pool(name="psA", bufs=2, space="PSUM") as psA,
            tc.tile_pool(name="psS", bufs=2, space="PSUM") as psS,
            tc.tile_pool(name="psO", bufs=2, space="PSUM") as psO,
            tc.tile_pool(name="psT", bufs=2, space="PSUM") as psT,
        ):
            win_sb = st.tile([128, 8 * D], F32R, tag="win")
            for et in range(8):
                for hf in range(2):
                    nc.sync.dma_start(
                        win_sb[:, et * D + hf * 512:et * D + (hf + 1) * 512],
                        win_d[et * 128:(et + 1) * 128, hf * 512:(hf + 1) * 512])

            def qry_dma(b, blk_i):
                q0 = blk_i * MQ
                t = qry_pool.tile([128, 8 * MQ], F32R, tag="qry")
                for dt in range(8):
                    nc.sync.dma_start(
                        t[:, dt * MQ:(dt + 1) * MQ],
                        qT_d[b, dt * 128:(dt + 1) * 128, q0:q0 + MQ])
                return t

            def qry16_dma(b, blk_i, pool=None, tag="q16"):
                q0 = blk_i * MQ
                pool = pool or q16_pool
                t = pool.tile([128, 8 * MQ], FP16, tag=tag)
                for dt in range(8):
                    nc.sync.dma_start(
                        t[:, dt * MQ:(dt + 1) * MQ],
                        qT16_d[b, dt * 128:(dt + 1) * 128, q0:q0 + MQ])
                return t

            def load_cT(b):
                """Scores-side context; its slot frees at the previous
                batch's cw build, so it can prefetch a block early."""
                NQB, NKN, NKT = params[b]
                S = sum(_k_chunks(NKT))
                cT_sb = ctx_pool.tile([128, 8 * Lk], F32R, tag="cT")
                for et in range(8):
                    nc.sync.dma_start(
                        cT_sb[:, et * Lk:et * Lk + S],
                        cT_d[b, et * 128:(et + 1) * 128, :S])
                return dict(cT=cT_sb)

            def load_masks(b, ctx):
                """Mask tiles: slots free at the previous batch's last
                scores chain, so these load at the crossing itself."""
                NQB, NKN, NKT = params[b]
                S = sum(_k_chunks(NKT))
                kmin_sb = ctx_pool.tile([128, Lk], F32, tag="kmin")
                nc.sync.dma_start(kmin_sb[:, :S], kmin_d[b, :, :S])
                qmin_sb = ctx_pool.tile([128, Lq // 128], F32, tag="qmin")
                nc.sync.dma_start(qmin_sb[:], qmin_d[b])
                q01_sb = ctx_pool.tile([128, Lq // 128], F32, tag="q01")
                nc.sync.dma_start(q01_sb[:], q01_d[b])
                ctx.update(kmin=kmin_sb, qmin=qmin_sb, q01=q01_sb)

            def load_ctx_late(b, ctx):
                """Tiles whose slots are released only by the previous
                batch's last mix: emitted after that mix to avoid blocking
                the in-order DMA queue behind an unsatisfiable wait."""
                NQB, NKN, NKT = params[b]
                cn_sb = ctx_pool.tile([128, 8 * D], FP16, tag="cn")
                for kt in range(NKT):
                    nc.sync.dma_start(cn_sb[:, kt * D:(kt + 1) * D],
                                      cn_d[b, kt * 128:(kt + 1) * 128, :])
                m01_sb = ctx_pool.tile([1, Lq], FP16, tag="m01")
                nc.sync.dma_start(m01_sb[:], m01_d[b:b + 1, :])
                mean_sb = ctx_pool.tile([1, D], FP16, tag="mean")
                nc.sync.dma_start(mean_sb[:], mean_d[b:b + 1, :])
                cb_sb = ctx_pool.tile([1, D], FP16, tag="cb")
                nc.sync.dma_start(cb_sb[:], cb_d[b:b + 1, :])
                ctx.update(cn=cn_sb, m01=m01_sb, mean=mean_sb, cb=cb_sb)

            def cw_build(b, ctx):
                """cw[d, k] = sum_e W_in[e, d] * contextT[e, k] (f32r)."""
                NQB, NKN, NKT = params[b]
                cw_sb = ctx_pool.tile([128, 8 * Lk], F32R, tag="cw")
                for dt in range(8):
                    off = 0
                    for w in _k_chunks(NKT):
                        ps = psS.tile([128, 512], F32, tag="psS")
                        for et in range(8):
                            nc.tensor.matmul(
                                ps[:, :w],
                                win_sb[:, et * D + dt * 128:et * D + (dt + 1) * 128],
                                ctx["cT"][:, et * Lk + off:et * Lk + off + w],
                                start=(et == 0), stop=(et == 7))
                        nc.vector.tensor_copy(
                            cw_sb[:, dt * Lk + off:dt * Lk + off + w],
                            ps[:, :w])
                        off += w
                ctx["cw"] = cw_sb

            # one [128, 1024] tile of exact -1e9 for skipped score regions,
            # plus a [1, 128] ones row for the constant-output rank-1 matmul
            const_sb = st.tile([128, 512], F32, tag="const")
            nc.vector.memset(const_sb[:], NEG)
            ones_sb = st.tile([1, 128], FP16, tag="ones")
            nc.vector.memset(ones_sb[:], 1.0)

            def scores_softmax(b, blk_i, qry_sb, ctx):
                """Masked scores -> DRAM; softmax weights -> ew tile."""
                NQB, NKN, NKT = params[b]
                chunks = _k_chunks(NKT)
                S = sum(chunks)
                NCH = len(chunks)
                q0 = blk_i * MQ
                ew_sb = ew_pool.tile([128, 2 * Lk], FP16, tag="ew")
                for h in range(2):
                    jt = blk_i * 2 + h
                    rows = slice(q0 + h * 128, q0 + (h + 1) * 128)
                    stt = stats_pool.tile([128, 8], F32, tag="stats")
                    sm_n = []
                    off = 0
                    for n, w in enumerate(chunks):
                        ps = psS.tile([128, 512], F32, tag="psS")
                        for dt in range(8):
                            nc.tensor.matmul(
                                ps[:, :w],
                                qry_sb[:, dt * MQ + h * 128:dt * MQ + (h + 1) * 128],
                                ctx["cw"][:, dt * Lk + off:dt * Lk + off + w],
                                start=(dt == 0), stop=(dt == 7))
                        sm = sm_pool.tile([128, 512], F32, tag="sm")
                        sm_n.append((sm, off, w))
                        nc.vector.tensor_tensor(
                            sm[:, :w], ps[:, :w], ctx["kmin"][:, off:off + w],
                            op=mybir.AluOpType.min)
                        nc.vector.tensor_scalar_min(
                            sm[:, :w], sm[:, :w], ctx["qmin"][:, jt:jt + 1])
                        nc.sync.dma_start(sc_d[b, rows, off:off + w], sm[:, :w])
                        nc.vector.reduce_max(
                            stt[:, n:n + 1], sm[:, :w],
                            axis=mybir.AxisListType.X, negate=True)
                        off += w
                    # fill the uncomputed score columns with exact -1e9
                    coff = S
                    while coff < Lk:
                        cw_w = min(512, Lk - coff)
                        nc.sync.dma_start(sc_d[b, rows, coff:coff + cw_w],
                                          const_sb[:, :cw_w])
                        coff += cw_w
                    if NCH == 1:
                        negm = stt[:, 0:1]
                    else:
                        nc.vector.tensor_tensor(
                            stt[:, 2:3], stt[:, 0:1], stt[:, 1:2],
                            op=mybir.AluOpType.min)
                        negm = stt[:, 2:3]
                    for n, (sm, off, w) in enumerate(sm_n):
                        nc.scalar.activation(
                            ew_sb[:, h * Lk + off:h * Lk + off + w],
                            sm[:, :w],
                            mybir.ActivationFunctionType.Exp,
                            bias=negm, scale=1.0,
                            accum_out=stt[:, 3 + n:4 + n])
                    if NCH == 1:
                        ssum = stt[:, 3:4]
                    else:
                        nc.vector.tensor_tensor(
                            stt[:, 5:6], stt[:, 3:4], stt[:, 4:5],
                            op=mybir.AluOpType.add)
                        ssum = stt[:, 5:6]
                    nc.vector.reciprocal(stt[:, 6:7], ssum)
                    if NKT == 8:
                        scale = stt[:, 6:7]
                    else:
                        # zero the weights of fully-masked query rows; their
                        # uniform mix is re-added as a rank-1 term in stage 3
                        nc.vector.tensor_tensor(
                            stt[:, 7:8], stt[:, 6:7], ctx["q01"][:, jt:jt + 1],
                            op=mybir.AluOpType.mult)
                        scale = stt[:, 7:8]
                    nc.vector.tensor_scalar_mul(
                        ew_sb[:, h * Lk:h * Lk + S],
                        ew_sb[:, h * Lk:h * Lk + S],
                        scale)
                return ew_sb

            def transposes(b, ew_sb, ident):
                NQB, NKN, NKT = params[b]
                wt_sb = wm_pool.tile([128, 8 * MQ], FP16, tag="wm")
                for kt in range(NKT):
                    pt = psT.tile([128, MQ], FP16, tag="psT")
                    for h in range(2):
                        nc.tensor.transpose(
                            pt[:, h * 128:(h + 1) * 128],
                            ew_sb[:, h * Lk + kt * 128:h * Lk + (kt + 1) * 128],
                            ident[:])
                    nc.vector.tensor_copy(wt_sb[:, kt * MQ:(kt + 1) * MQ], pt[:])
                return wt_sb

            def mix_stage(b, blk_i, wt_sb, ctx):
                NQB, NKN, NKT = params[b]
                q0 = blk_i * MQ
                mixT_sb = wm_pool.tile([128, 8 * MQ], FP16, tag="wm")
                for dt in range(8):
                    pm = psA.tile([128, MQ], F32, tag="psA")
                    for kt in range(NKT):
                        nc.tensor.matmul(
                            pm[:],
                            ctx["cn"][:, kt * D + dt * 128:kt * D + (dt + 1) * 128],
                            wt_sb[:, kt * MQ:(kt + 1) * MQ],
                            start=(kt == 0), stop=(kt == NKT - 1 and NKT == 8))
                        # rank-1: uniform context-mean for fully-masked queries
                    if NKT < 8:
                        nc.tensor.matmul(
                            pm[:],
                            ctx["mean"][0:1, dt * 128:(dt + 1) * 128],
                            ctx["m01"][0:1, q0:q0 + MQ],
                            start=False, stop=True)
                    nc.vector.tensor_copy(mixT_sb[:, dt * MQ:(dt + 1) * MQ], pm[:])
                return mixT_sb

            def out_stage(b, blk_i, q16_sb, mixT_sb):
                q0 = blk_i * MQ
                for h in range(2):
                    rows = slice(q0 + h * 128, q0 + (h + 1) * 128)
                    for n in range(2):
                        po = psO.tile([128, 512], F32, tag="psO")
                        for dt in range(8):
                            nc.tensor.matmul(
                                po[:],
                                mixT_sb[:, dt * MQ + h * 128:dt * MQ + (h + 1) * 128],
                                wo1_sb[:, dt * D + n * 512:dt * D + (n + 1) * 512],
                                start=(dt == 0), stop=False)
                        for dt in range(8):
                            nc.tensor.matmul(
                                po[:],
                                q16_sb[:, dt * MQ + h * 128:dt * MQ + (h + 1) * 128],
                                wf_sb[:, dt * D + n * 512:dt * D + (n + 1) * 512],
                                start=False, stop=(dt == 7))
                        ot = ot_pool.tile([128, 512], F32, tag="ot")
                        nc.scalar.activation(
                            ot[:], po[:], mybir.ActivationFunctionType.Tanh)
                        nc.sync.dma_start(out_d[b, rows, n * 512:(n + 1) * 512],
                                          ot[:])

            def skipped_block(b, blk_i, ctx):
                """q-block past every query length in the slot: scores are
                all -1e9; out = tanh(query@Wfused + mean_ctx@Wo1)."""
                q0 = blk_i * MQ
                q16_sb = qry16_dma(b, blk_i, pool=q16s_pool, tag="q16s")
                for h in range(2):
                    rows = slice(q0 + h * 128, q0 + (h + 1) * 128)
                    for n4 in range(2):
                        nc.sync.dma_start(
                            sc_d[b, rows, n4 * 512:(n4 + 1) * 512], const_sb[:])
                    for n in range(2):
                        po = psO.tile([128, 512], F32, tag="psO")
                        for dt in range(8):
                            nc.tensor.matmul(
                                po[:],
                                q16_sb[:, dt * MQ + h * 128:dt * MQ + (h + 1) * 128],
                                wf_sb[:, dt * D + n * 512:dt * D + (n + 1) * 512],
                                start=(dt == 0), stop=False)
                        nc.tensor.matmul(
                            po[:], ones_sb[0:1, :],
                            ctx["cb"][0:1, n * 512:(n + 1) * 512],
                            start=False, stop=True)
                        ot = ot_pool.tile([128, 512], F32, tag="ot")
                        nc.scalar.activation(
                            ot[:], po[:], mybir.ActivationFunctionType.Tanh)
                        nc.sync.dma_start(out_d[b, rows, n * 512:(n + 1) * 512],
                                          ot[:])

            # ---- prologue ---------------------------------------------
            qry0 = qry_dma(0, 0)
            ctx0 = load_cT(0)
            load_masks(0, ctx0)
            load_ctx_late(0, ctx0)

            wo1_sb = st.tile([128, 8 * D], FP16, tag="wo1")
            for ct in range(8):
                nc.sync.dma_start(wo1_sb[:, ct * D:(ct + 1) * D],
                                  wo1_d[ct * 128:(ct + 1) * 128, :])
            wf_sb = st.tile([128, 8 * D], FP16, tag="wf")
            for ct in range(8):
                nc.sync.dma_start(wf_sb[:, ct * D:(ct + 1) * D],
                                  wf_d[ct * 128:(ct + 1) * 128, :])
            ident = st.tile([128, 128], FP16, tag="ident")
            make_identity(nc, ident[:])

            cw_build(0, ctx0)

            # flattened computed-block sequence with one-block lookahead:
            # scores(next) is emitted before transposes(cur) so the PE has
            # work while the softmax chain runs on DVE/ACT.
            seq = [(b, i) for b in range(BPC) for i in range(params[b][0])]
            cur_ctx = {0: ctx0}
            q16_0 = qry16_dma(0, 0)
            pend = (0, 0, q16_0, scores_softmax(0, 0, qry0, ctx0))
            for idx in range(len(seq)):
                b, i = seq[idx]
                _, _, q16_sb, ew_sb = pend
                ctx = cur_ctx[b]
                # prefetch the next batch's scores-side context one block
                # before the crossing so the cw build never waits on DMA
                if idx + 2 < len(seq) and seq[idx + 2][0] != seq[idx + 1][0]:
                    fb = seq[idx + 2][0]
                    cur_ctx[fb] = load_cT(fb)
                nxt = seq[idx + 1] if idx + 1 < len(seq) else None
                if nxt is not None:
                    nb, ni = nxt
                    if nb != b:
                        if nb not in cur_ctx:
                            cur_ctx[nb] = load_cT(nb)
                        load_masks(nb, cur_ctx[nb])
                        cw_build(nb, cur_ctx[nb])
                    nqry = qry_dma(nb, ni)
                    sew = scores_softmax(nb, ni, nqry, cur_ctx[nb])
                    pend = (nb, ni, qry16_dma(nb, ni), sew)
                wt_sb = transposes(b, ew_sb, ident)
                mixT_sb = mix_stage(b, i, wt_sb, ctx)
                out_stage(b, i, q16_sb, mixT_sb)
                if nxt is None or nxt[0] != b:
                    for si in range(params[b][0], NBLK):
                        skipped_block(b, si, ctx)
                if nxt is not None and nxt[0] != b:
                    load_ctx_late(nxt[0], cur_ctx[nxt[0]])

    nc.compile()
    return nc


def kernel(query, context, query_lengths, context_lengths, W_in, W_out):
    slots, params = _assign_slots(np.asarray(query_lengths),
                                  np.asarray(context_lengths))
    if _cache.get("params") != params:
        _cache["nc"] = _build_program(params)
        _cache["params"] = params
    nc = _cache["nc"]

    # batch order: core c processes batches [slots[0][c], slots[1][c], ...]
    perm = np.array(slots)                       # [BPC, N_CORES]
    flat = perm.T.reshape(-1)                    # core-major batch order

    query = np.asarray(query, dtype=np.float32)
    context = np.asarray(context, dtype=np.float32)
    ql = np.asarray(query_lengths).astype(np.int64)
    cl = np.asarray(context_lengths).astype(np.int64)

    qT = np.ascontiguousarray(query.transpose(0, 2, 1))
    qT16 = qT.astype(np.float16)
    cT = np.ascontiguousarray(context.transpose(0, 2, 1))
    cn = context.astype(np.float16)
    win = np.ascontiguousarray(W_in, dtype=np.float32)
    woT = np.ascontiguousarray(W_out.T, dtype=np.float32)
    wo1 = woT[:D].astype(np.float16)
    wf = (W_in.astype(np.float64).T @ woT[D:].astype(np.float64)).astype(np.float16)
    mean_c = context.astype(np.float64).mean(axis=1)           # [B, D]
    cb = (mean_c @ woT[:D].astype(np.float64)).astype(np.float16)
    mean_c = mean_c.astype(np.float16)

    k_idx = np.arange(Lk)
    q_idx = np.arange(Lq)
    kvalid = k_idx[None, :] < cl[:, None]
    qvalid = q_idx[None, :] < ql[:, None]
    kmin = np.where(kvalid, np.float32(BIG), np.float32(NEG)).astype(np.float32)
    qmin = np.where(qvalid, np.float32(BIG), np.float32(NEG)).astype(np.float32)
    q01 = qvalid.astype(np.float32)
    m01 = (~qvalid).astype(np.float16)
    kmin_rep = np.ascontiguousarray(
        np.broadcast_to(kmin[:, None, :], (B, 128, Lk)))
    qmin_til = np.ascontiguousarray(
        qmin.reshape(B, Lq // 128, 128).transpose(0, 2, 1))
    q01_til = np.ascontiguousarray(
        q01.reshape(B, Lq // 128, 128).transpose(0, 2, 1))

    in_maps = []
    for c in range(N_CORES):
        s = flat[c * BPC:(c + 1) * BPC]
        in_maps.append({
            "qT": np.ascontiguousarray(qT[s]),
            "qT16": np.ascontiguousarray(qT16[s]),
            "cT": np.ascontiguousarray(cT[s]),
            "cn": np.ascontiguousarray(cn[s]),
            "win": win, "wo1": wo1, "wf": wf,
            "kmin": np.ascontiguousarray(kmin_rep[s]),
            "qmin": np.ascontiguousarray(qmin_til[s]),
            "q01": np.ascontiguousarray(q01_til[s]),
            "m01": np.ascontiguousarray(m01[s]),
            "mean": np.ascontiguousarray(mean_c[s]),
            "cb": np.ascontiguousarray(cb[s]),
        })

    res = bass_utils.run_bass_kernel_spmd(nc, in_maps, core_ids=list(range(N_CORES)))
    _cache["last_result"] = res

    out = np.empty((B, Lq, D), dtype=np.float32)
    scores = np.empty((B, Lq, Lk), dtype=np.float32)
    for c in range(N_CORES):
        s = flat[c * BPC:(c + 1) * BPC]
        out[s] = res.results[c]["out"]
        scores[s] = res.results[c]["sc"]
    return out, scores


_program_cache = _cache  # test.py compatibility
